# revision 1
# baseline (speedup 1.0000x reference)
"""DeepSTN on 8 Trainium2 NeuronCores — v2.

Baseline structure (replicated convs, OC-sharded GEMM, 1 AllGather/iter) plus:
- plus_conv weight in fp8e4 (2-D Floyd-Steinberg error-diffused, x256 scale),
  streamed per iteration (16.8MB vs 33.5MB bf16) -> DMA fully hidden.
- GEMM in DoubleRow fp8 perf mode with z2 split into fp8 hi + fp8 residual:
  256 DR instructions accumulating into one PSUM group (2x fewer cycles
  than bf16, numerics within tolerance).
- 3x3 convs tap-paired: padded images duplicated into partitions 64-127
  shifted by one column (one SBUF->SBUF DMA), so 2 taps contract per
  matmul: 6 instructions/sample instead of 9.
"""
import numpy as np
import ml_dtypes

B, H, W = 32, 32, 16
HW = H * W            # 512
NC = 8                # cores
NFF = B * HW          # 16384 free elems per channel (full batch)
C = 64                # cpt channels
RP = 4                # ResPlus iterations
KCH = 256             # GEMM k-chunks of 128
OSH = 512             # output shard (4096 / 8)
EPS = 1e-5
PADR, PADC = H + 2, W + 2   # 34, 18
PB = PADR * PADC            # 612 per sample
PBT = B * PB                # 19584

_HANDLE = {}

import os as _os
DEFAULT_CFG = (int(_os.environ.get("KRP", "4")),
               _os.environ.get("KCOLL", "1") == "1",
               _os.environ.get("KGEMM", "1") == "1",
               _os.environ.get("KCONV", "1") == "1",
               int(_os.environ.get("KWG", "16")))


def _aff(g, b, m, v):
    s = g / np.sqrt(v + EPS)
    return s.astype(np.float32), (b - m * s).astype(np.float32)


def _bf(a):
    return np.asarray(a, np.float32).astype(ml_dtypes.bfloat16)


def _f8(a):
    return np.asarray(a, np.float32).astype(ml_dtypes.float8_e4m3)


def _diffuse2d(Wt):
    """Error-feedback fp8 quantization of W [64*HW, NO], 2-D FS per
    channel over the 32x16 spatial plane, serpentine scan."""
    q = np.empty(Wt.shape, ml_dtypes.float8_e4m3)
    NO = Wt.shape[1]
    for c in range(64):
        blk = Wt[c * HW:(c + 1) * HW].reshape(H, W, NO).astype(np.float32)
        for y in range(H):
            sgn = 1 if y % 2 == 0 else -1
            xs = range(W) if y % 2 == 0 else range(W - 1, -1, -1)
            for x in xs:
                v = blk[y, x]
                q8 = v.astype(ml_dtypes.float8_e4m3)
                e = v - q8.astype(np.float32)
                q[c * HW + y * W + x] = q8
                if 0 <= x + sgn < W:
                    blk[y, x + sgn] += e * (7 / 16)
                if y + 1 < H:
                    if 0 <= x - sgn < W:
                        blk[y + 1, x - sgn] += e * (3 / 16)
                    blk[y + 1, x] += e * (5 / 16)
                    if 0 <= x + sgn < W:
                        blk[y + 1, x + sgn] += e * (1 / 16)
    return q


def _build_nc(cfg=None):
    import concourse.bacc as bacc
    import concourse.mybir as mybir
    import concourse.tile as tile

    cfg = tuple(cfg) if cfg else DEFAULT_CFG
    if len(cfg) == 4:
        cfg = cfg + (16,)
    KRP, KCOLL, KGEMM, KCONV, WG = cfg
    f32, bf16, fp8 = mybir.dt.float32, mybir.dt.bfloat16, mybir.dt.float8e4
    DR = mybir.MatmulPerfMode.DoubleRow
    nc = bacc.Bacc("TRN2", target_bir_lowering=False, debug=False, num_devices=NC)
    RG = [list(range(NC))]

    def din(name, shape, dt=f32):
        return nc.dram_tensor(name, shape, dt, kind="ExternalInput")

    xin = {}
    xin["x_c"] = din("x_c", [6, NFF])
    xin["x_p"] = din("x_p", [8, NFF])
    xin["x_t"] = din("x_t", [8, NFF])
    xin["x_poi"] = din("x_poi", [12, NFF])
    xin["x_tm"] = din("x_tm", [31, NFF])
    for name, shape in [
        ("wcp", [14, 128]), ("wtpoi", [20, 73]), ("ones1", [1, 12]),
        ("wtm_T", [31, 28]), ("wtf_T", [28, 1]),
        ("W1a_T", [128, 64]), ("W1b_T", [73, 64]),
        ("w3p", [128, 6 * 56]), ("w4p", [128, 6 * 64]), ("W2_T", [64, 2]),
        ("ident", [64, 64]),
    ]:
        xin[name] = din(name, shape, bf16)
    for name, shape in [
        ("bcp", [128, 1]), ("btpoi", [73, 1]), ("btm", [28, 1]), ("btf", [1, 1]),
        ("b1", [64, 1]), ("sa", [64, 1]), ("ta", [64, 1]), ("sa16", [64, 1]),
        ("s56", [56, 1]), ("bz1", [56, 1]), ("s8", [8, 1]), ("t8", [8, 1]),
        ("b4", [64, 1]), ("b2", [2, 1]), ("bplus", [B, OSH]),
    ]:
        xin[name] = din(name, shape, f32)
    xin["wplus"] = din("wplus", [128, KCH, OSH], fp8)

    out_ext = nc.dram_tensor("out", [B, 2, HW], f32, kind="ExternalOutput")

    HC = OSH // 2  # column half: spatial rows 0:16 | 16:32 of each zm map
    ag_in = [nc.dram_tensor(f"ag_in{i}", [B, HC], bf16)
             for i in range(2 * KRP)]
    ag_out = [nc.dram_tensor(f"ag_out{i}", [NC * B, HC], bf16,
                             addr_space="Shared") for i in range(2 * KRP)]

    Relu = mybir.ActivationFunctionType.Relu
    Tanh = mybir.ActivationFunctionType.Tanh

    with tile.TileContext(nc) as tc:
        with (
            tc.tile_pool(name="wsm", bufs=1) as wsm,
            tc.tile_pool(name="act", bufs=1) as act,
            tc.tile_pool(name="wst", bufs=2) as wst,
        ):
            # ---- small weights ----
            wt = {}
            for name in ["wcp", "wtpoi", "ones1", "wtm_T", "wtf_T",
                         "W1a_T", "W1b_T", "W2_T", "ident",
                         "w3p", "w4p",
                         "bcp", "btpoi", "btm", "btf", "b1", "sa", "ta", "sa16",
                         "s56", "bz1", "s8", "t8", "b4", "b2", "bplus"]:
                t = wsm.tile(list(xin[name].shape), xin[name].dtype, tag=name)
                nc.sync.dma_start(t[:], xin[name][:])
                wt[name] = t

            # ---- head: x -> cpt [64, NFF] f32 (full batch, chunked) ----
            cpt = act.tile([C, NFF], f32, tag="cpt")
            with (
                tc.tile_pool(name="hx", bufs=2) as hx,
                tc.tile_pool(name="hps", bufs=1, space="PSUM") as hps,
                tc.tile_pool(name="hsb", bufs=2) as hsb,
            ):
                GSZ = 2048
                for grp in range(NFF // GSZ):
                    gsl = slice(grp * GSZ, (grp + 1) * GSZ)
                    xcp = hx.tile([14, GSZ], bf16, tag="xcp")
                    xtp = hx.tile([20, GSZ], bf16, tag="xtp")
                    xtm = hx.tile([31, GSZ], bf16, tag="x_tm")
                    # casting DMAs into block-diag stationaries' partition slots
                    nc.gpsimd.dma_start(xcp[0:6, :], xin["x_c"][:, gsl])
                    nc.gpsimd.dma_start(xcp[6:14, :], xin["x_p"][:, gsl])
                    nc.gpsimd.dma_start(xtp[0:12, :], xin["x_poi"][:, gsl])
                    nc.gpsimd.dma_start(xtp[12:20, :], xin["x_t"][:, gsl])
                    nc.gpsimd.dma_start(xtm[:], xin["x_tm"][:, gsl])
                    NQ = 512
                    for qq in range(GSZ // NQ):
                        sl = slice(qq * NQ, (qq + 1) * NQ)
                        osl = slice(grp * GSZ + qq * NQ, grp * GSZ + (qq + 1) * NQ)
                        p_tm = hps.tile([28, NQ], f32, tag="h_tm")
                        nc.tensor.matmul(p_tm[:], wt["wtm_T"][:], xtm[:, sl],
                                         start=True, stop=True)
                        tx = hsb.tile([28, NQ], bf16, tag="h_tx")
                        nc.scalar.activation(tx[:], p_tm[:], Relu, bias=wt["btm"][:])
                        p_tf = hps.tile([1, NQ], f32, tag="h_tf")
                        nc.tensor.matmul(p_tf[:], wt["wtf_T"][:], tx[:],
                                         start=True, stop=True)
                        tx2 = hsb.tile([1, NQ], bf16, tag="h_tx2")
                        nc.scalar.activation(tx2[:], p_tf[:], Relu, bias=wt["btf"][:])
                        # poi *= broadcast(tx2), in place
                        p_bc = hps.tile([12, NQ], f32, tag="h_bc")
                        nc.tensor.matmul(p_bc[:], wt["ones1"][:], tx2[:],
                                         start=True, stop=True)
                        nc.vector.tensor_mul(xtp[0:12, sl], xtp[0:12, sl],
                                             p_bc[:])

                        p1 = hps.tile([128, NQ], f32, tag="h_p1")
                        nc.tensor.matmul(p1[:], wt["wcp"][:], xcp[:, sl],
                                         start=True, stop=True)
                        p2 = hps.tile([73, NQ], f32, tag="h_p2")
                        nc.tensor.matmul(p2[:], wt["wtpoi"][:], xtp[:, sl],
                                         start=True, stop=True)
                        y1 = hsb.tile([128, NQ], bf16, tag="h_y1")
                        nc.scalar.activation(y1[:], p1[:], Relu, bias=wt["bcp"][:])
                        y2 = hsb.tile([73, NQ], bf16, tag="h_y2")
                        nc.scalar.activation(y2[:], p2[:], Relu, bias=wt["btpoi"][:])
                        p_c1 = hps.tile([64, NQ], f32, tag="h_c1")
                        nc.tensor.matmul(p_c1[:], wt["W1a_T"][:], y1[:],
                                         start=True, stop=False)
                        nc.tensor.matmul(p_c1[:], wt["W1b_T"][:], y2[:],
                                         start=False, stop=True)
                        nc.vector.tensor_scalar_add(cpt[:, osl], p_c1[:], wt["b1"][:])

            # ---- iteration-phase pools ----
            with (
                tc.tile_pool(name="itr", bufs=1) as itr,
                tc.tile_pool(name="zb", bufs=2) as zb,
                tc.tile_pool(name="pg", bufs=1, space="PSUM") as pgp,
                tc.tile_pool(name="pcv", bufs=2, space="PSUM") as pcv,
                tc.tile_pool(name="ptr", bufs=2, space="PSUM") as ptrp,
            ):
                z1pad = itr.tile([128, PBT], bf16, tag="z1pad")
                nc.vector.memset(z1pad[:], 0.0)
                z3pad = itr.tile([128, PBT], bf16, tag="z3pad")
                nc.vector.memset(z3pad[:], 0.0)
                zt_hi = itr.tile([128, KCH * 32], fp8, tag="zt_hi")
                zt_lo = itr.tile([128, KCH * 32], fp8, tag="zt_lo")
                stage = itr.tile([B, OSH], bf16, tag="stage")

                z1pad_v = z1pad[:].rearrange("c (b y x) -> c b y x", b=B, y=PADR)
                z3pad_v = z3pad[:].rearrange("c (b y x) -> c b y x", b=B, y=PADR)
                cpt_v = cpt[:].rearrange("c (b y x) -> c b y x", b=B, y=H)
                zthv = zt_hi[:].rearrange("p (b sb c) -> p sb c b", b=B, sb=4)
                ztlv = zt_lo[:].rearrange("p (b sb c) -> p sb c b", b=B, sb=4)
                wpv = xin["wplus"].ap()

                for it in range(KRP):
                    # z2 split: zt_hi = fp8(relu(16*sa*cpt)^T), zt_lo = residual
                    for g in range(8):
                        sg = slice(g * GSZ, (g + 1) * GSZ)
                        z2h = zb.tile([C, GSZ], bf16, tag="z2h")
                        nc.scalar.activation(z2h[:], cpt[:, sg], Relu,
                                             scale=wt["sa16"][:])
                        ptb = ptrp.tile([128, 1024], bf16, tag="tr")
                        for t16 in range(16):
                            nc.tensor.transpose(
                                ptb[:, t16 * 64:(t16 + 1) * 64],
                                z2h[:, t16 * 128:(t16 + 1) * 128], wt["ident"][:])
                        csl = slice(g * 1024, (g + 1) * 1024)
                        nc.vector.tensor_copy(zt_hi[:, csl], ptb[:])
                        nc.vector.tensor_sub(zt_lo[:, csl], ptb[:], zt_hi[:, csl])

                    # z1pad interior write (overlaps GEMM on scalar engine)
                    if KCONV:
                        for b in range(B):
                            nc.scalar.activation(
                                z1pad_v[0:64, b, 1:33, 1:17], cpt_v[:, b],
                                Relu, scale=wt["sa"][:], bias=wt["ta"][:])
                        nc.sync.dma_start(z1pad[64:128, 0:PBT - 1],
                                          z1pad[0:64, 1:PBT])

                    # GEMM in two column halves; each half's AllGather
                    # launches as soon as that half's columns complete, so
                    # both collectives fly while z1-conv runs on PE.
                    pg = pgp.tile([B, OSH], f32, tag="g")
                    if not KGEMM:
                        nc.vector.memset(pg[:], 0.0)
                    for half in range(2):
                        csl = slice(half * HC, (half + 1) * HC)
                        for jg in range(KCH // WG if KGEMM else 0):
                            wtile = wst.tile([128, WG * HC], fp8, tag="w")
                            nc.sync.dma_start(
                                wtile[:].rearrange("p (k o) -> p k o", k=WG),
                                wpv[:, jg * WG:(jg + 1) * WG, csl])
                            wtv = wtile[:].rearrange("p (k o) -> p k o", k=WG)
                            for pr in range(WG // 2):
                                j0 = jg * WG + 2 * pr
                                c_i, sb_i = j0 // 4, j0 % 4
                                rhs = wtv[:, 2 * pr:2 * pr + 2, :]
                                nc.tensor.matmul(
                                    pg[:, csl],
                                    zthv[:, sb_i:sb_i + 2, c_i:c_i + 1, :],
                                    rhs, start=(j0 == 0), stop=False,
                                    perf_mode=DR)
                                nc.tensor.matmul(
                                    pg[:, csl],
                                    ztlv[:, sb_i:sb_i + 2, c_i:c_i + 1, :],
                                    rhs, start=False, stop=(j0 == KCH - 2),
                                    perf_mode=DR)
                        nc.vector.tensor_add(stage[:, csl], pg[:, csl],
                                             wt["bplus"][:, csl])
                        nc.sync.dma_start(ag_in[2 * it + half][:],
                                          stage[:, csl])
                        if KCOLL:
                            nc.gpsimd.collective_compute(
                                "AllGather", mybir.AluOpType.bypass,
                                replica_groups=RG,
                                ins=[ag_in[2 * it + half][:].opt()],
                                outs=[ag_out[2 * it + half][:].opt()])

                    # z1 conv (paired taps) - overlaps the AllGather
                    for b in range(B if KCONV else 0):
                        pz = pcv.tile([56, HW], f32, tag="cv")
                        for s in range(3):
                            nc.tensor.matmul(
                                pz[:], wt["w3p"][:, s * 56:(s + 1) * 56],
                                z1pad_v[:, b, s:s + 32, 0:16],
                                start=(s == 0), stop=False)
                        for s in range(3):
                            nc.tensor.matmul(
                                pz[:], wt["w3p"][0:64, (3 + s) * 56:(4 + s) * 56],
                                z1pad_v[0:64, b, s:s + 32, 2:18],
                                start=False, stop=(s == 2))
                        nc.scalar.activation(
                            z3pad_v[0:56, b, 1:33, 1:17], pz[:],
                            Relu, scale=wt["s56"][:], bias=wt["bz1"][:])
                    if KCONV:
                        nc.sync.dma_start(z3pad[64:120, 0:PBT - 1],
                                          z3pad[0:56, 1:PBT])

                    # gathered z2map half -> z3pad rows (relu/affine staged at
                    # base 0, DMA'd into partitions 56:64 + shifted 120:128)
                    def zm_half(half):
                        yr = (slice(1, 17), slice(17, 33))[half]
                        for zg_i in range(8):
                            zmg = zb.tile([8, 4 * HC], bf16, tag="zmg")
                            nc.sync.dma_start(
                                zmg[:].rearrange("j (b s) -> j b s", b=4),
                                ag_out[2 * it + half].ap().rearrange(
                                    "(j b) s -> j b s",
                                    j=NC)[:, zg_i * 4:zg_i * 4 + 4])
                            zmrel = zb.tile([8, 4 * HC], bf16, tag="zmrel")
                            nc.scalar.activation(zmrel[:], zmg[:], Relu,
                                                 scale=wt["s8"][:],
                                                 bias=wt["t8"][:])
                            zmrel_v = zmrel[:].rearrange(
                                "j (b y x) -> j b y x", b=4, y=16)
                            for bb in range(4):
                                b = zg_i * 4 + bb
                                nc.sync.dma_start(
                                    z3pad_v[56:64, b, yr, 1:17], zmrel_v[:, bb])
                                nc.sync.dma_start(
                                    z3pad_v[120:128, b, yr, 0:16],
                                    zmrel_v[:, bb])

                    # conv2 split into top rows 0:14 (needs zm half 0 only)
                    # and bottom rows 14:32 (needs both halves) + residual
                    zm_half(0)
                    for b in range(B if KCONV else 0):
                        pc2t = pcv.tile([64, 288], f32, tag="cv")
                        pc2 = pc2t[:, 0:224]
                        for s in range(3):
                            nc.tensor.matmul(
                                pc2[:], wt["w4p"][:, s * 64:(s + 1) * 64],
                                z3pad_v[:, b, s:s + 14, 0:16],
                                start=(s == 0), stop=False)
                        for s in range(3):
                            nc.tensor.matmul(
                                pc2[:],
                                wt["w4p"][0:64, (3 + s) * 64:(4 + s) * 64],
                                z3pad_v[0:64, b, s:s + 14, 2:18],
                                start=False, stop=(s == 2))
                        sl = slice(b * HW, b * HW + 224)
                        nc.vector.tensor_add(cpt[:, sl], pc2[:], cpt[:, sl])
                        nc.vector.tensor_scalar_add(cpt[:, sl], cpt[:, sl],
                                                    wt["b4"][:])
                    zm_half(1)
                    for b in range(B if KCONV else 0):
                        pc2 = pcv.tile([64, 288], f32, tag="cv")
                        for s in range(3):
                            nc.tensor.matmul(
                                pc2[:], wt["w4p"][:, s * 64:(s + 1) * 64],
                                z3pad_v[:, b, 14 + s:32 + s, 0:16],
                                start=(s == 0), stop=False)
                        for s in range(3):
                            nc.tensor.matmul(
                                pc2[:],
                                wt["w4p"][0:64, (3 + s) * 64:(4 + s) * 64],
                                z3pad_v[0:64, b, 14 + s:32 + s, 2:18],
                                start=False, stop=(s == 2))
                        sl = slice(b * HW + 224, (b + 1) * HW)
                        nc.vector.tensor_add(cpt[:, sl], pc2[:], cpt[:, sl])
                        nc.vector.tensor_scalar_add(cpt[:, sl], cpt[:, sl],
                                                    wt["b4"][:])

                # ---- tail ----
                with tc.tile_pool(name="tps", bufs=2, space="PSUM") as tps:
                    for q in range(NFF // 512):
                        sl = slice(q * 512, (q + 1) * 512)
                        rq = zb.tile([C, 512], bf16, tag="z2h")
                        nc.scalar.activation(rq[:], cpt[:, sl], Relu)
                        po = tps.tile([2, 512], f32, tag="t_o")
                        nc.tensor.matmul(po[:], wt["W2_T"][:], rq[:],
                                         start=True, stop=True)
                        oq = zb.tile([2, 512], f32, tag="oq")
                        nc.scalar.activation(oq[:], po[:], Tanh, bias=wt["b2"][:])
                        nc.sync.dma_start(out_ext.ap()[q], oq[:])

    nc.compile()
    return nc


def _prep_inputs(inputs):
    """Host-side preprocessing -> list of 8 per-core input dicts."""
    ii = {k: np.asarray(v, np.float32) if np.asarray(v).dtype == np.float32
          else np.asarray(v) for k, v in inputs.items()}

    s1, t1 = _aff(ii["bn1_g"], ii["bn1_b"], ii["bn1_m"], ii["bn1_v"])
    sa, ta = _aff(ii["rp_bn1_g"], ii["rp_bn1_b"], ii["rp_bn1_m"], ii["rp_bn1_v"])
    sb_, tb_ = _aff(ii["rp_bn2_g"], ii["rp_bn2_b"], ii["rp_bn2_m"], ii["rp_bn2_v"])
    sc_, tc_ = _aff(ii["bn2_g"], ii["bn2_b"], ii["bn2_m"], ii["bn2_v"])

    conv1_2d = ii["conv1_w"][:, :, 0, 0]
    W1p = conv1_2d * s1[None, :]
    b1p = conv1_2d @ t1 + ii["conv1_b"]
    conv2_2d = ii["conv2_w"][:, :, 0, 0]
    W2p = conv2_2d * sc_[None, :]
    b2p = conv2_2d @ tc_ + ii["conv2_b"]

    # paired conv stationaries: slots 0-2 = taps (dy,0)|(dy,1) stacked on
    # partitions 0-63|64-127; slots 3-5 = taps (dy,2) on partitions 0-63.
    def pair_taps(wf, o):  # wf [O, I, 3, 3] -> [128, 6*o]
        out = np.zeros((128, 6 * o), np.float32)
        for dy in range(3):
            out[0:wf.shape[1], dy * o:(dy + 1) * o] = wf[:, :, dy, 0].T
            out[64:64 + wf.shape[1], dy * o:(dy + 1) * o] = wf[:, :, dy, 1].T
            out[0:wf.shape[1], (3 + dy) * o:(4 + dy) * o] = wf[:, :, dy, 2].T
        return out

    w3p = pair_taps(ii["rp_conv1_w"], 56)
    # conv2 with true 64 channels: rows = 56 z1-ch then 8 zm-ch
    w4p = pair_taps(ii["rp_conv2_w"], 64)

    wcp = np.zeros((14, 128), np.float32)
    wcp[0:6, 0:64] = ii["convc_w"][:, :, 0, 0].T
    wcp[6:14, 64:128] = ii["convp_w"][:, :, 0, 0].T
    wtpoi = np.zeros((20, 73), np.float32)
    wtpoi[0:12, 64:73] = ii["poi_w"][:, :, 0, 0].T
    wtpoi[12:20, 0:64] = ii["convt_w"][:, :, 0, 0].T

    xl = ii["x"].transpose(1, 0, 2, 3).reshape(65, NFF)
    base = {
        "x_c": np.ascontiguousarray(xl[0:6]),
        "x_p": np.ascontiguousarray(xl[6:14]),
        "x_t": np.ascontiguousarray(xl[14:22]),
        "x_poi": np.ascontiguousarray(xl[22:34]),
        "x_tm": np.ascontiguousarray(xl[34:65]),
        "wcp": _bf(wcp), "wtpoi": _bf(wtpoi), "ones1": _bf(np.ones((1, 12))),
        "wtm_T": _bf(ii["tm_w"][:, :, 0, 0].T),
        "wtf_T": _bf(ii["tf_w"][:, :, 0, 0].T),
        "W1a_T": _bf(W1p[:, :128].T),
        "W1b_T": _bf(W1p[:, 128:].T),
        "w3p": _bf(w3p), "w4p": _bf(w4p),
        "W2_T": _bf(W2p.T), "ident": _bf(np.eye(64)),
        "bcp": np.concatenate([ii["convc_b"], ii["convp_b"]])[:, None].astype(np.float32),
        "btpoi": np.concatenate([ii["convt_b"], ii["poi_b"]])[:, None].astype(np.float32),
        "btm": ii["tm_b"][:, None], "btf": ii["tf_b"][:, None],
        "b1": b1p[:, None].astype(np.float32), "sa": sa[:, None], "ta": ta[:, None],
        "sa16": (16.0 * sa)[:, None].astype(np.float32),
        "s56": sb_[:56, None],
        "bz1": (sb_[:56] * ii["rp_conv1_b"] + tb_[:56])[:, None].astype(np.float32),
        "s8": (sb_[56:] / 4096.0)[:, None].astype(np.float32),
        "t8": tb_[56:, None],
        "b4": ii["rp_conv2_b"][:, None], "b2": b2p[:, None].astype(np.float32),
    }

    plus_wf = ii["plus_w"].reshape(8 * HW, C * HW)
    W8 = _diffuse2d(plus_wf.T * 256.0)  # [64*HW, 4096] fp8
    ta_flat = np.repeat(ta, HW)
    in_maps = []
    for c in range(NC):
        m = dict(base)
        Wsh = plus_wf[c * OSH:(c + 1) * OSH]
        bias_eff = (ii["plus_b"][c * OSH:(c + 1) * OSH] + Wsh @ ta_flat)
        m["bplus"] = np.broadcast_to(4096.0 * bias_eff,
                                     (B, OSH)).astype(np.float32).copy()
        # wplus[p, j, o] = W8[k = j*128 + p, o_shard]
        W8c = W8[:, c * OSH:(c + 1) * OSH]  # [32768, 512]
        m["wplus"] = np.ascontiguousarray(
            W8c.reshape(KCH, 128, OSH).transpose(1, 0, 2))
        in_maps.append(m)
    return in_maps


def _build_sharded(nc):
    import jax
    import numpy as _np
    from jax.sharding import Mesh, PartitionSpec
    from jax.experimental.shard_map import shard_map
    import concourse.mybir as mybir
    from concourse.bass2jax import (_bass_exec_p, install_neuronx_cc_hook,
                                    partition_id_tensor)

    install_neuronx_cc_hook()
    partition_name = nc.partition_id_tensor.name if nc.partition_id_tensor else None
    in_names, out_names, out_avals, zero_outs = [], [], [], []
    for alloc in nc.m.functions[0].allocations:
        if not isinstance(alloc, mybir.MemoryLocationSet):
            continue
        name = alloc.memorylocations[0].name
        if alloc.kind == "ExternalInput":
            if name != partition_name:
                in_names.append(name)
        elif alloc.kind == "ExternalOutput":
            shape = tuple(alloc.tensor_shape)
            dtype = mybir.dt.np(alloc.dtype)
            out_avals.append(jax.core.ShapedArray(shape, dtype))
            out_names.append(name)
            zero_outs.append(_np.zeros(shape, dtype))
    n_params = len(in_names)
    n_outs = len(out_avals)
    all_in_names = list(in_names) + list(out_names)
    if partition_name is not None:
        all_in_names.append(partition_name)
    donate = tuple(range(n_params, n_params + n_outs))

    def _body(*args):
        operands = list(args)
        if partition_name is not None:
            operands.append(partition_id_tensor())
        outs = _bass_exec_p.bind(
            *operands, out_avals=tuple(out_avals), in_names=tuple(all_in_names),
            out_names=tuple(out_names), lowering_input_output_aliases=(),
            sim_require_finite=True, sim_require_nnan=True, nc=nc)
        return tuple(outs)

    devices = jax.devices()[:NC]
    mesh = Mesh(_np.asarray(devices), ("core",))
    in_specs = (PartitionSpec("core"),) * (n_params + n_outs)
    out_specs = (PartitionSpec("core"),) * n_outs
    fn = jax.jit(
        shard_map(_body, mesh=mesh, in_specs=in_specs, out_specs=out_specs,
                  check_rep=False),
        donate_argnums=donate, keep_unused=True)
    return dict(fn=fn, in_names=in_names, out_names=out_names,
                out_avals=out_avals, zero_outs=zero_outs)


def get_compiled(cfg=None):
    key = tuple(cfg) if cfg else DEFAULT_CFG
    if len(key) == 4:
        key = key + (16,)
    if key not in _HANDLE:
        nc = _build_nc(key)
        _HANDLE[key] = _build_sharded(nc)
    return _HANDLE[key]


def stage_inputs(in_maps, cfg=None):
    import jax
    import numpy as _np
    from jax.sharding import Mesh, NamedSharding, PartitionSpec
    h = get_compiled(cfg)
    mesh = Mesh(_np.asarray(jax.devices()[:NC]), ("core",))
    sh = NamedSharding(mesh, PartitionSpec("core"))
    concat_in = [_np.concatenate([_np.asarray(in_maps[c][name]) for c in range(NC)],
                                 axis=0) for name in h["in_names"]]
    return [jax.device_put(a, sh) for a in concat_in]


def run_staged(staged, cfg=None):
    import jax
    import numpy as _np
    h = get_compiled(cfg)
    concat_zeros = [_np.zeros((NC * z.shape[0], *z.shape[1:]), z.dtype)
                    for z in h["zero_outs"]]
    out_arrs = h["fn"](*staged, *concat_zeros)
    jax.block_until_ready(out_arrs)
    return [
        {name: _np.asarray(out_arrs[i]).reshape(NC, *h["out_avals"][i].shape)[c]
         for i, name in enumerate(h["out_names"])}
        for c in range(NC)
    ]


def run_spmd(in_maps, cfg=None):
    import jax
    import numpy as _np
    h = get_compiled(cfg)
    concat_in = [_np.concatenate([_np.asarray(in_maps[c][name]) for c in range(NC)],
                                 axis=0) for name in h["in_names"]]
    concat_zeros = [_np.zeros((NC * z.shape[0], *z.shape[1:]), z.dtype)
                    for z in h["zero_outs"]]
    out_arrs = h["fn"](*concat_in, *concat_zeros)
    jax.block_until_ready(out_arrs)
    return [
        {name: _np.asarray(out_arrs[i]).reshape(NC, *h["out_avals"][i].shape)[c]
         for i, name in enumerate(h["out_names"])}
        for c in range(NC)
    ]


def kernel(**inputs):
    in_maps = _prep_inputs(inputs)
    results = run_spmd(in_maps)
    return results[0]["out"].reshape(B, 2, H, W).astype(np.float32)



# revision 10
# speedup vs baseline: 1.2547x; 1.2547x over previous
"""DeepSTN on 8 Trainium2 NeuronCores — v3.

v2 baseline: replicated convs, OC-sharded fp8 DoubleRow GEMM with hi+lo
error-split (2 passes over the weight stream), 1 AllGather/iter.

v3 changes:
- Merged hi|lo stationary: hi and lo z2^T packed into one 128-wide DR
  stationary [128, 2, 64] -> both products accumulate during a SINGLE
  weight stream; PSUM [64, 512] rows 0:32 = hi, 32:64 = lo, folded with
  one vector add. Halves GEMM instruction count (512 -> 256/iter).
- Batch-sharded convs/head/tail: each core computes convs + transposes
  + zt emit only for its own 4 samples (b in [4c, 4c+4)); the fp8 z2^T
  (hi|lo) is AllGathered per iteration (2x 128KB/rank, A=k-chunks
  j%4<2, B=j%4>=2, pipelined so GEMM starts on A while B flies).
- plus_conv fp8 weight cached in SBUF (128KB/partition) across the 4
  ResPlus iterations: HBM streamed once instead of 4x.
"""
import numpy as np
import ml_dtypes

B, H, W = 32, 32, 16
HW = H * W            # 512
NC = 8                # cores
NB = B // NC          # 4 own samples per core
NFO = NB * HW         # 2048 own free elems
C = 64                # cpt channels
KCH = 256             # GEMM k-chunks of 128
OSH = 512             # output shard (4096 / 8)
HC = OSH // 2         # GEMM column half
EPS = 1e-5
PADR, PADC = H + 2, W + 2   # 34, 18
PB = PADR * PADC            # 612 per sample
PBO = NB * PB               # 2448 own padded cols

_HANDLE = {}

import os as _os
DEFAULT_CFG = (int(_os.environ.get("KRP", "4")),
               _os.environ.get("KCOLL", "1") == "1",
               _os.environ.get("KGEMM", "1") == "1",
               _os.environ.get("KCONV", "1") == "1",
               int(_os.environ.get("KWG", "16")))


def _aff(g, b, m, v):
    s = g / np.sqrt(v + EPS)
    return s.astype(np.float32), (b - m * s).astype(np.float32)


def _bf(a):
    return np.asarray(a, np.float32).astype(ml_dtypes.bfloat16)


def _diffuse2d(Wt):
    """Error-feedback fp8 quantization of W [64*HW, NO], 2-D FS per
    channel over the 32x16 spatial plane, serpentine scan."""
    q = np.empty(Wt.shape, ml_dtypes.float8_e4m3)
    NO = Wt.shape[1]
    for c in range(64):
        blk = Wt[c * HW:(c + 1) * HW].reshape(H, W, NO).astype(np.float32)
        for y in range(H):
            sgn = 1 if y % 2 == 0 else -1
            xs = range(W) if y % 2 == 0 else range(W - 1, -1, -1)
            for x in xs:
                v = blk[y, x]
                q8 = v.astype(ml_dtypes.float8_e4m3)
                e = v - q8.astype(np.float32)
                q[c * HW + y * W + x] = q8
                if 0 <= x + sgn < W:
                    blk[y, x + sgn] += e * (7 / 16)
                if y + 1 < H:
                    if 0 <= x - sgn < W:
                        blk[y + 1, x - sgn] += e * (3 / 16)
                    blk[y + 1, x] += e * (5 / 16)
                    if 0 <= x + sgn < W:
                        blk[y + 1, x + sgn] += e * (1 / 16)
    return q


def _build_nc(cfg=None):
    import concourse.bacc as bacc
    import concourse.mybir as mybir
    import concourse.tile as tile

    cfg = tuple(cfg) if cfg else DEFAULT_CFG
    if len(cfg) == 4:
        cfg = cfg + (16,)
    KRP, KCOLL, KGEMM, KCONV, WG = cfg
    f32, bf16, fp8 = mybir.dt.float32, mybir.dt.bfloat16, mybir.dt.float8e4
    DR = mybir.MatmulPerfMode.DoubleRow
    nc = bacc.Bacc("TRN2", target_bir_lowering=False, debug=False, num_devices=NC)
    RG = [list(range(NC))]

    def din(name, shape, dt=f32):
        return nc.dram_tensor(name, shape, dt, kind="ExternalInput")

    xin = {}
    xin["x_c"] = din("x_c", [6, NFO])
    xin["x_p"] = din("x_p", [8, NFO])
    xin["x_t"] = din("x_t", [8, NFO])
    xin["x_poi"] = din("x_poi", [12, NFO])
    xin["x_tm"] = din("x_tm", [31, NFO])
    for name, shape in [
        ("wcp", [14, 128]), ("wtpoi", [20, 73]), ("ones1", [1, 12]),
        ("wtm_T", [31, 28]), ("wtf_T", [28, 1]),
        ("W1a_T", [128, 64]), ("W1b_T", [73, 64]),
        ("w3p", [128, 6 * 56]), ("w4p", [128, 6 * 64]), ("W2_T", [64, 2]),
        ("ident", [64, 64]),
    ]:
        xin[name] = din(name, shape, bf16)
    for name, shape in [
        ("bcp", [128, 1]), ("btpoi", [73, 1]), ("btm", [28, 1]), ("btf", [1, 1]),
        ("b1", [64, 1]), ("sa", [64, 1]), ("ta", [64, 1]), ("sa16", [64, 1]),
        ("s56", [56, 1]), ("bz1", [56, 1]), ("s8", [8, 1]), ("t8", [8, 1]),
        ("b4", [64, 1]), ("b2", [2, 1]), ("bplus", [B, OSH]),
    ]:
        xin[name] = din(name, shape, f32)
    xin["wplus"] = din("wplus", [128, KCH, OSH], fp8)

    out_ext = nc.dram_tensor("out", [NB, 2, HW], f32, kind="ExternalOutput")

    # zt AllGather staging: slot it feeds GEMM(it). Contribution per core:
    # [128, 1024] fp8 laid out (hl, bb, col128): hi of A-k-chunks | lo.
    agzA_in = [nc.dram_tensor(f"agzAi{i}", [128, 1024], fp8) for i in range(KRP)]
    agzA_out = [nc.dram_tensor(f"agzAo{i}", [NC * 128, 1024], fp8,
                               addr_space="Shared") for i in range(KRP)]
    agzB_in = [nc.dram_tensor(f"agzBi{i}", [128, 1024], fp8) for i in range(KRP)]
    agzB_out = [nc.dram_tensor(f"agzBo{i}", [NC * 128, 1024], fp8,
                               addr_space="Shared") for i in range(KRP)]
    # z2-map AllToAll (2 column halves per iteration): core j sends its
    # OC-channel rows [4k,4k+4) to core k; rows [4j,4j+4) of the output
    # hold channel j for this core's own 4 samples.
    agm_in = [nc.dram_tensor(f"agmi{i}", [B, HC], bf16) for i in range(2 * KRP)]
    agm_out = [nc.dram_tensor(f"agmo{i}", [B, HC], bf16)
               for i in range(2 * KRP)]

    Relu = mybir.ActivationFunctionType.Relu
    Tanh = mybir.ActivationFunctionType.Tanh

    with tile.TileContext(nc) as tc:
        with (
            tc.tile_pool(name="wsm", bufs=1) as wsm,
            tc.tile_pool(name="act", bufs=1) as act,
        ):
            # ---- small weights ----
            wt = {}
            for name in ["wcp", "wtpoi", "ones1", "wtm_T", "wtf_T",
                         "W1a_T", "W1b_T", "W2_T", "ident",
                         "w3p", "w4p",
                         "bcp", "btpoi", "btm", "btf", "b1", "sa", "ta", "sa16",
                         "s56", "bz1", "s8", "t8", "b4", "b2", "bplus"]:
                t = wsm.tile(list(xin[name].shape), xin[name].dtype, tag=name)
                nc.sync.dma_start(t[:], xin[name][:])
                wt[name] = t

            # ---- W cache: full per-core fp8 shard in SBUF ----
            wsb = act.tile([128, KCH * OSH], fp8, tag="wsb")
            wsbv = wsb[:].rearrange("p (j o) -> p j o", j=KCH)
            for jg in range(KCH // WG):
                nc.sync.dma_start(wsbv[:, jg * WG:(jg + 1) * WG, :],
                                  xin["wplus"].ap()[:, jg * WG:(jg + 1) * WG, :])

            # ---- head: own x cols -> cpt [64, NFO] f32 ----
            cpt = act.tile([C, NFO], f32, tag="cpt")
            with (
                tc.tile_pool(name="hx", bufs=1) as hx,
                tc.tile_pool(name="hps", bufs=1, space="PSUM") as hps,
                tc.tile_pool(name="hsb", bufs=2) as hsb,
            ):
                xcp = hx.tile([14, NFO], bf16, tag="xcp")
                xtp = hx.tile([20, NFO], bf16, tag="xtp")
                xtm = hx.tile([31, NFO], bf16, tag="x_tm")
                nc.gpsimd.dma_start(xcp[0:6, :], xin["x_c"][:])
                nc.gpsimd.dma_start(xcp[6:14, :], xin["x_p"][:])
                nc.gpsimd.dma_start(xtp[0:12, :], xin["x_poi"][:])
                nc.gpsimd.dma_start(xtp[12:20, :], xin["x_t"][:])
                nc.gpsimd.dma_start(xtm[:], xin["x_tm"][:])
                NQ = 512
                for qq in range(NFO // NQ):
                    sl = slice(qq * NQ, (qq + 1) * NQ)
                    p_tm = hps.tile([28, NQ], f32, tag="h_tm")
                    nc.tensor.matmul(p_tm[:], wt["wtm_T"][:], xtm[:, sl],
                                     start=True, stop=True)
                    tx = hsb.tile([28, NQ], bf16, tag="h_tx")
                    nc.scalar.activation(tx[:], p_tm[:], Relu, bias=wt["btm"][:])
                    p_tf = hps.tile([1, NQ], f32, tag="h_tf")
                    nc.tensor.matmul(p_tf[:], wt["wtf_T"][:], tx[:],
                                     start=True, stop=True)
                    tx2 = hsb.tile([1, NQ], bf16, tag="h_tx2")
                    nc.scalar.activation(tx2[:], p_tf[:], Relu, bias=wt["btf"][:])
                    p_bc = hps.tile([12, NQ], f32, tag="h_bc")
                    nc.tensor.matmul(p_bc[:], wt["ones1"][:], tx2[:],
                                     start=True, stop=True)
                    nc.vector.tensor_mul(xtp[0:12, sl], xtp[0:12, sl], p_bc[:])

                    p1 = hps.tile([128, NQ], f32, tag="h_p1")
                    nc.tensor.matmul(p1[:], wt["wcp"][:], xcp[:, sl],
                                     start=True, stop=True)
                    p2 = hps.tile([73, NQ], f32, tag="h_p2")
                    nc.tensor.matmul(p2[:], wt["wtpoi"][:], xtp[:, sl],
                                     start=True, stop=True)
                    y1 = hsb.tile([128, NQ], bf16, tag="h_y1")
                    nc.scalar.activation(y1[:], p1[:], Relu, bias=wt["bcp"][:])
                    y2 = hsb.tile([73, NQ], bf16, tag="h_y2")
                    nc.scalar.activation(y2[:], p2[:], Relu, bias=wt["btpoi"][:])
                    p_c1 = hps.tile([64, NQ], f32, tag="h_c1")
                    nc.tensor.matmul(p_c1[:], wt["W1a_T"][:], y1[:],
                                     start=True, stop=False)
                    nc.tensor.matmul(p_c1[:], wt["W1b_T"][:], y2[:],
                                     start=False, stop=True)
                    nc.vector.tensor_scalar_add(cpt[:, sl], p_c1[:], wt["b1"][:])

            # ---- iteration-phase pools ----
            with (
                tc.tile_pool(name="itr", bufs=1) as itr,
                tc.tile_pool(name="zb", bufs=2) as zb,
                tc.tile_pool(name="pg", bufs=1, space="PSUM") as pgp,
                tc.tile_pool(name="pcv", bufs=2, space="PSUM") as pcv,
                tc.tile_pool(name="ptr", bufs=2, space="PSUM") as ptrp,
            ):
                z1pad = itr.tile([128, PBO], bf16, tag="z1pad")
                nc.vector.memset(z1pad[:], 0.0)
                z3pad = itr.tile([128, PBO], bf16, tag="z3pad")
                nc.vector.memset(z3pad[:], 0.0)
                # gathered z2^T stationaries, col = (2b+hl... see below)
                # ztA col = m*128 + sb*64 + c   (m = hl*32 + b, sb in {0,1})
                # ztB col = m*128 + sb*64 + c   (sb in {2,3} local 0,1)
                ztA = itr.tile([128, 8192], fp8, tag="ztA")
                ztB = itr.tile([128, 8192], fp8, tag="ztB")
                sthi = itr.tile([128, 1024], fp8, tag="sthi")
                stlo = itr.tile([128, 1024], fp8, tag="stlo")
                stage = itr.tile([B, OSH], bf16, tag="stage")
                tmpf = itr.tile([B, OSH], f32, tag="tmpf")

                z1pad_v = z1pad[:].rearrange("c (b y x) -> c b y x", b=NB, y=PADR)
                z3pad_v = z3pad[:].rearrange("c (b y x) -> c b y x", b=NB, y=PADR)
                cpt_v = cpt[:].rearrange("c (b y x) -> c b y x", b=NB, y=H)
                # stationary views: [p, sb, c, m]
                ztAv = ztA[:].rearrange("p (m sb c) -> p sb c m", m=64, sb=2)
                ztBv = ztB[:].rearrange("p (m sb c) -> p sb c m", m=64, sb=2)
                sthi_v = sthi[:].rearrange("p (bb sc) -> p bb sc", bb=NB)
                stlo_v = stlo[:].rearrange("p (bb sc) -> p bb sc", bb=NB)

                def emit_sample(bb):
                    """cpt own sample bb -> hi/lo fp8 staging (transposed)."""
                    sl = slice(bb * HW, (bb + 1) * HW)
                    z2h = zb.tile([C, HW], bf16, tag="z2h")
                    nc.scalar.activation(z2h[:], cpt[:, sl], Relu,
                                         scale=wt["sa16"][:])
                    ptb = ptrp.tile([128, 256], bf16, tag="tr")
                    for sb in range(4):
                        nc.tensor.transpose(
                            ptb[:, sb * 64:(sb + 1) * 64],
                            z2h[:, sb * 128:(sb + 1) * 128], wt["ident"][:])
                    nc.vector.tensor_copy(sthi_v[:, bb, :], ptb[:])
                    nc.vector.tensor_sub(stlo_v[:, bb, :], ptb[:],
                                         sthi_v[:, bb, :])

                def emit_flush(slot):
                    """staging -> ag_in DRAM + launch zt AllGathers."""
                    agA = agzA_in[slot].ap().rearrange(
                        "p (hl bb cc) -> p hl bb cc", hl=2, bb=NB)
                    agB = agzB_in[slot].ap().rearrange(
                        "p (hl bb cc) -> p hl bb cc", hl=2, bb=NB)
                    nc.sync.dma_start(agA[:, 0], sthi_v[:, :, 0:128])
                    nc.sync.dma_start(agA[:, 1], stlo_v[:, :, 0:128])
                    nc.sync.dma_start(agB[:, 0], sthi_v[:, :, 128:256])
                    nc.sync.dma_start(agB[:, 1], stlo_v[:, :, 128:256])
                    if KCOLL:
                        nc.gpsimd.collective_compute(
                            "AllGather", mybir.AluOpType.bypass,
                            replica_groups=RG,
                            ins=[agzA_in[slot][:].opt()],
                            outs=[agzA_out[slot][:].opt()])
                        nc.gpsimd.collective_compute(
                            "AllGather", mybir.AluOpType.bypass,
                            replica_groups=RG,
                            ins=[agzB_in[slot][:].opt()],
                            outs=[agzB_out[slot][:].opt()])

                def consume_zt(slot, which):
                    """ag_out -> zt tile. which: 0=A, 1=B."""
                    zt = (ztA, ztB)[which]
                    ago = (agzA_out, agzB_out)[which][slot]
                    # zt col = (hl*32+b)*128 + inner ; b = cc*4+bb
                    dst = zt[:].rearrange("p (hl cc k) -> p hl cc k",
                                          hl=2, cc=NC)
                    src = ago.ap().rearrange("(cc p) (hl k) -> p hl cc k",
                                             cc=NC, hl=2)
                    nc.sync.dma_start(dst, src)

                # head emit -> slot 0
                for bb in range(NB):
                    emit_sample(bb)
                emit_flush(0)

                for it in range(KRP):
                    # ---- conv1 (z1) for own samples ----
                    if KCONV:
                        for bb in range(NB):
                            nc.scalar.activation(
                                z1pad_v[0:64, bb, 1:33, 1:17], cpt_v[:, bb],
                                Relu, scale=wt["sa"][:], bias=wt["ta"][:])
                        nc.sync.dma_start(z1pad[64:128, 0:PBO - 1],
                                          z1pad[0:64, 1:PBO])
                        for bb in range(NB):
                            pz = pcv.tile([56, HW], f32, tag="cv")
                            for s in range(3):
                                nc.tensor.matmul(
                                    pz[:], wt["w3p"][:, s * 56:(s + 1) * 56],
                                    z1pad_v[:, bb, s:s + 32, 0:16],
                                    start=(s == 0), stop=False)
                            for s in range(3):
                                nc.tensor.matmul(
                                    pz[:], wt["w3p"][0:64, (3 + s) * 56:(4 + s) * 56],
                                    z1pad_v[0:64, bb, s:s + 32, 2:18],
                                    start=False, stop=(s == 2))
                            nc.scalar.activation(
                                z3pad_v[0:56, bb, 1:33, 1:17], pz[:],
                                Relu, scale=wt["s56"][:], bias=wt["bz1"][:])
                        nc.sync.dma_start(z3pad[64:120, 0:PBO - 1],
                                          z3pad[0:56, 1:PBO])

                    # ---- GEMM: 2 OC halves x (A-pairs then B-pairs) ----
                    consume_zt(it, 0)
                    pg = pgp.tile([64, OSH], f32, tag="g")
                    if not KGEMM:
                        nc.vector.memset(pg[:], 0.0)
                    for half in range(2):
                        csl = slice(half * HC, (half + 1) * HC)
                        if KGEMM:
                            for cc in range(64):
                                nc.tensor.matmul(
                                    pg[:, csl], ztAv[:, :, cc:cc + 1, :],
                                    wsbv[:, 4 * cc:4 * cc + 2, csl],
                                    start=(cc == 0), stop=False, perf_mode=DR)
                            if half == 0:
                                consume_zt(it, 1)
                            for cc in range(64):
                                nc.tensor.matmul(
                                    pg[:, csl], ztBv[:, :, cc:cc + 1, :],
                                    wsbv[:, 4 * cc + 2:4 * cc + 4, csl],
                                    start=False, stop=(cc == 63), perf_mode=DR)
                        nc.vector.tensor_add(tmpf[:, csl], pg[32:64, csl],
                                             wt["bplus"][:, csl])
                        nc.vector.tensor_add(stage[:, csl], pg[0:32, csl],
                                             tmpf[:, csl])
                        nc.sync.dma_start(agm_in[2 * it + half][:],
                                          stage[:, csl])
                        if KCOLL:
                            nc.gpsimd.collective_compute(
                                "AllToAll", mybir.AluOpType.bypass,
                                replica_groups=RG,
                                ins=[agm_in[2 * it + half][:].opt()],
                                outs=[agm_out[2 * it + half][:].opt()])

                    # gathered z2-map half -> z3pad rows (own samples only)
                    def zm_half(half):
                        yr = (slice(1, 17), slice(17, 33))[half]
                        zmg = zb.tile([8, NB * HC], bf16, tag="zmg")
                        nc.sync.dma_start(
                            zmg[:].rearrange("j (b s) -> j b s", b=NB),
                            agm_out[2 * it + half].ap().rearrange(
                                "(j b) s -> j b s", j=NC))
                        zmrel = zb.tile([8, NB * HC], bf16, tag="zmrel")
                        nc.scalar.activation(zmrel[:], zmg[:], Relu,
                                             scale=wt["s8"][:],
                                             bias=wt["t8"][:])
                        zmrel_v = zmrel[:].rearrange(
                            "j (b y x) -> j b y x", b=NB, y=16)
                        for bb in range(NB):
                            nc.sync.dma_start(
                                z3pad_v[56:64, bb, yr, 1:17], zmrel_v[:, bb])
                            nc.sync.dma_start(
                                z3pad_v[120:128, bb, yr, 0:16],
                                zmrel_v[:, bb])

                    # conv2 rows 0:14 (needs zm half 0), rows 14:32 (both)
                    zm_half(0)
                    for bb in range(NB if KCONV else 0):
                        pc2t = pcv.tile([64, 288], f32, tag="cv")
                        pc2 = pc2t[:, 0:224]
                        for s in range(3):
                            nc.tensor.matmul(
                                pc2[:], wt["w4p"][:, s * 64:(s + 1) * 64],
                                z3pad_v[:, bb, s:s + 14, 0:16],
                                start=(s == 0), stop=False)
                        for s in range(3):
                            nc.tensor.matmul(
                                pc2[:],
                                wt["w4p"][0:64, (3 + s) * 64:(4 + s) * 64],
                                z3pad_v[0:64, bb, s:s + 14, 2:18],
                                start=False, stop=(s == 2))
                        sl = slice(bb * HW, bb * HW + 224)
                        nc.vector.tensor_add(cpt[:, sl], pc2[:], cpt[:, sl])
                        nc.vector.tensor_scalar_add(cpt[:, sl], cpt[:, sl],
                                                    wt["b4"][:])
                    zm_half(1)
                    for bb in range(NB if KCONV else 0):
                        pc2 = pcv.tile([64, 288], f32, tag="cv")
                        for s in range(3):
                            nc.tensor.matmul(
                                pc2[:], wt["w4p"][:, s * 64:(s + 1) * 64],
                                z3pad_v[:, bb, 14 + s:32 + s, 0:16],
                                start=(s == 0), stop=False)
                        for s in range(3):
                            nc.tensor.matmul(
                                pc2[:],
                                wt["w4p"][0:64, (3 + s) * 64:(4 + s) * 64],
                                z3pad_v[0:64, bb, 14 + s:32 + s, 2:18],
                                start=False, stop=(s == 2))
                        sl = slice(bb * HW + 224, (bb + 1) * HW)
                        nc.vector.tensor_add(cpt[:, sl], pc2[:], cpt[:, sl])
                        nc.vector.tensor_scalar_add(cpt[:, sl], cpt[:, sl],
                                                    wt["b4"][:])
                        if it + 1 < KRP:
                            emit_sample(bb)
                    if it + 1 < KRP:
                        emit_flush(it + 1)

                # ---- tail ----
                with tc.tile_pool(name="tps", bufs=2, space="PSUM") as tps:
                    for q in range(NB):
                        sl = slice(q * 512, (q + 1) * 512)
                        rq = zb.tile([C, 512], bf16, tag="z2h")
                        nc.scalar.activation(rq[:], cpt[:, sl], Relu)
                        po = tps.tile([2, 512], f32, tag="t_o")
                        nc.tensor.matmul(po[:], wt["W2_T"][:], rq[:],
                                         start=True, stop=True)
                        oq = zb.tile([2, 512], f32, tag="oq")
                        nc.scalar.activation(oq[:], po[:], Tanh, bias=wt["b2"][:])
                        nc.sync.dma_start(out_ext.ap()[q], oq[:])

    nc.compile()
    return nc


def _prep_inputs(inputs):
    """Host-side preprocessing -> list of 8 per-core input dicts."""
    ii = {k: np.asarray(v, np.float32) if np.asarray(v).dtype == np.float32
          else np.asarray(v) for k, v in inputs.items()}

    s1, t1 = _aff(ii["bn1_g"], ii["bn1_b"], ii["bn1_m"], ii["bn1_v"])
    sa, ta = _aff(ii["rp_bn1_g"], ii["rp_bn1_b"], ii["rp_bn1_m"], ii["rp_bn1_v"])
    sb_, tb_ = _aff(ii["rp_bn2_g"], ii["rp_bn2_b"], ii["rp_bn2_m"], ii["rp_bn2_v"])
    sc_, tc_ = _aff(ii["bn2_g"], ii["bn2_b"], ii["bn2_m"], ii["bn2_v"])

    conv1_2d = ii["conv1_w"][:, :, 0, 0]
    W1p = conv1_2d * s1[None, :]
    b1p = conv1_2d @ t1 + ii["conv1_b"]
    conv2_2d = ii["conv2_w"][:, :, 0, 0]
    W2p = conv2_2d * sc_[None, :]
    b2p = conv2_2d @ tc_ + ii["conv2_b"]

    # paired conv stationaries: slots 0-2 = taps (dy,0)|(dy,1) stacked on
    # partitions 0-63|64-127; slots 3-5 = taps (dy,2) on partitions 0-63.
    def pair_taps(wf, o):  # wf [O, I, 3, 3] -> [128, 6*o]
        out = np.zeros((128, 6 * o), np.float32)
        for dy in range(3):
            out[0:wf.shape[1], dy * o:(dy + 1) * o] = wf[:, :, dy, 0].T
            out[64:64 + wf.shape[1], dy * o:(dy + 1) * o] = wf[:, :, dy, 1].T
            out[0:wf.shape[1], (3 + dy) * o:(4 + dy) * o] = wf[:, :, dy, 2].T
        return out

    w3p = pair_taps(ii["rp_conv1_w"], 56)
    w4p = pair_taps(ii["rp_conv2_w"], 64)

    wcp = np.zeros((14, 128), np.float32)
    wcp[0:6, 0:64] = ii["convc_w"][:, :, 0, 0].T
    wcp[6:14, 64:128] = ii["convp_w"][:, :, 0, 0].T
    wtpoi = np.zeros((20, 73), np.float32)
    wtpoi[0:12, 64:73] = ii["poi_w"][:, :, 0, 0].T
    wtpoi[12:20, 0:64] = ii["convt_w"][:, :, 0, 0].T

    xl = ii["x"].transpose(1, 0, 2, 3).reshape(65, B * HW)
    base = {
        "wcp": _bf(wcp), "wtpoi": _bf(wtpoi), "ones1": _bf(np.ones((1, 12))),
        "wtm_T": _bf(ii["tm_w"][:, :, 0, 0].T),
        "wtf_T": _bf(ii["tf_w"][:, :, 0, 0].T),
        "W1a_T": _bf(W1p[:, :128].T),
        "W1b_T": _bf(W1p[:, 128:].T),
        "w3p": _bf(w3p), "w4p": _bf(w4p),
        "W2_T": _bf(W2p.T), "ident": _bf(np.eye(64)),
        "bcp": np.concatenate([ii["convc_b"], ii["convp_b"]])[:, None].astype(np.float32),
        "btpoi": np.concatenate([ii["convt_b"], ii["poi_b"]])[:, None].astype(np.float32),
        "btm": ii["tm_b"][:, None], "btf": ii["tf_b"][:, None],
        "b1": b1p[:, None].astype(np.float32), "sa": sa[:, None], "ta": ta[:, None],
        "sa16": (16.0 * sa)[:, None].astype(np.float32),
        "s56": sb_[:56, None],
        "bz1": (sb_[:56] * ii["rp_conv1_b"] + tb_[:56])[:, None].astype(np.float32),
        "s8": (sb_[56:] / 4096.0)[:, None].astype(np.float32),
        "t8": tb_[56:, None],
        "b4": ii["rp_conv2_b"][:, None], "b2": b2p[:, None].astype(np.float32),
    }

    plus_wf = ii["plus_w"].reshape(8 * HW, C * HW)
    W8 = _diffuse2d(plus_wf.T * 256.0)  # [64*HW, 4096] fp8
    ta_flat = np.repeat(ta, HW)
    in_maps = []
    for c in range(NC):
        m = dict(base)
        osl = slice(c * NFO, (c + 1) * NFO)
        m["x_c"] = np.ascontiguousarray(xl[0:6, osl])
        m["x_p"] = np.ascontiguousarray(xl[6:14, osl])
        m["x_t"] = np.ascontiguousarray(xl[14:22, osl])
        m["x_poi"] = np.ascontiguousarray(xl[22:34, osl])
        m["x_tm"] = np.ascontiguousarray(xl[34:65, osl])
        Wsh = plus_wf[c * OSH:(c + 1) * OSH]
        bias_eff = (ii["plus_b"][c * OSH:(c + 1) * OSH] + Wsh @ ta_flat)
        m["bplus"] = np.broadcast_to(4096.0 * bias_eff,
                                     (B, OSH)).astype(np.float32).copy()
        W8c = W8[:, c * OSH:(c + 1) * OSH]  # [32768, 512]
        m["wplus"] = np.ascontiguousarray(
            W8c.reshape(KCH, 128, OSH).transpose(1, 0, 2))
        in_maps.append(m)
    return in_maps


def _build_sharded(nc):
    import jax
    import numpy as _np
    from jax.sharding import Mesh, PartitionSpec
    from jax.experimental.shard_map import shard_map
    import concourse.mybir as mybir
    from concourse.bass2jax import (_bass_exec_p, install_neuronx_cc_hook,
                                    partition_id_tensor)

    install_neuronx_cc_hook()
    partition_name = nc.partition_id_tensor.name if nc.partition_id_tensor else None
    in_names, out_names, out_avals, zero_outs = [], [], [], []
    for alloc in nc.m.functions[0].allocations:
        if not isinstance(alloc, mybir.MemoryLocationSet):
            continue
        name = alloc.memorylocations[0].name
        if alloc.kind == "ExternalInput":
            if name != partition_name:
                in_names.append(name)
        elif alloc.kind == "ExternalOutput":
            shape = tuple(alloc.tensor_shape)
            dtype = mybir.dt.np(alloc.dtype)
            out_avals.append(jax.core.ShapedArray(shape, dtype))
            out_names.append(name)
            zero_outs.append(_np.zeros(shape, dtype))
    n_params = len(in_names)
    n_outs = len(out_avals)
    all_in_names = list(in_names) + list(out_names)
    if partition_name is not None:
        all_in_names.append(partition_name)
    donate = tuple(range(n_params, n_params + n_outs))

    def _body(*args):
        operands = list(args)
        if partition_name is not None:
            operands.append(partition_id_tensor())
        outs = _bass_exec_p.bind(
            *operands, out_avals=tuple(out_avals), in_names=tuple(all_in_names),
            out_names=tuple(out_names), lowering_input_output_aliases=(),
            sim_require_finite=True, sim_require_nnan=True, nc=nc)
        return tuple(outs)

    devices = jax.devices()[:NC]
    mesh = Mesh(_np.asarray(devices), ("core",))
    in_specs = (PartitionSpec("core"),) * (n_params + n_outs)
    out_specs = (PartitionSpec("core"),) * n_outs
    fn = jax.jit(
        shard_map(_body, mesh=mesh, in_specs=in_specs, out_specs=out_specs,
                  check_rep=False),
        donate_argnums=donate, keep_unused=True)
    return dict(fn=fn, in_names=in_names, out_names=out_names,
                out_avals=out_avals, zero_outs=zero_outs)


def get_compiled(cfg=None):
    key = tuple(cfg) if cfg else DEFAULT_CFG
    if len(key) == 4:
        key = key + (16,)
    if key not in _HANDLE:
        nc = _build_nc(key)
        _HANDLE[key] = _build_sharded(nc)
    return _HANDLE[key]


def stage_inputs(in_maps, cfg=None):
    import jax
    import numpy as _np
    from jax.sharding import Mesh, NamedSharding, PartitionSpec
    h = get_compiled(cfg)
    mesh = Mesh(_np.asarray(jax.devices()[:NC]), ("core",))
    sh = NamedSharding(mesh, PartitionSpec("core"))
    concat_in = [_np.concatenate([_np.asarray(in_maps[c][name]) for c in range(NC)],
                                 axis=0) for name in h["in_names"]]
    return [jax.device_put(a, sh) for a in concat_in]


def run_staged(staged, cfg=None):
    import jax
    import numpy as _np
    h = get_compiled(cfg)
    concat_zeros = [_np.zeros((NC * z.shape[0], *z.shape[1:]), z.dtype)
                    for z in h["zero_outs"]]
    out_arrs = h["fn"](*staged, *concat_zeros)
    jax.block_until_ready(out_arrs)
    return [
        {name: _np.asarray(out_arrs[i]).reshape(NC, *h["out_avals"][i].shape)[c]
         for i, name in enumerate(h["out_names"])}
        for c in range(NC)
    ]


def run_spmd(in_maps, cfg=None):
    import jax
    import numpy as _np
    h = get_compiled(cfg)
    concat_in = [_np.concatenate([_np.asarray(in_maps[c][name]) for c in range(NC)],
                                 axis=0) for name in h["in_names"]]
    concat_zeros = [_np.zeros((NC * z.shape[0], *z.shape[1:]), z.dtype)
                    for z in h["zero_outs"]]
    out_arrs = h["fn"](*concat_in, *concat_zeros)
    jax.block_until_ready(out_arrs)
    return [
        {name: _np.asarray(out_arrs[i]).reshape(NC, *h["out_avals"][i].shape)[c]
         for i, name in enumerate(h["out_names"])}
        for c in range(NC)
    ]


def kernel(**inputs):
    in_maps = _prep_inputs(inputs)
    results = run_spmd(in_maps)
    full = np.concatenate([results[c]["out"] for c in range(NC)], axis=0)
    return full.reshape(B, 2, H, W).astype(np.float32)


# revision 15
# speedup vs baseline: 1.3267x; 1.0574x over previous
"""DeepSTN on 8 Trainium2 NeuronCores — v3.

v2 baseline: replicated convs, OC-sharded fp8 DoubleRow GEMM with hi+lo
error-split (2 passes over the weight stream), 1 AllGather/iter.

v3 changes:
- Merged hi|lo stationary: hi and lo z2^T packed into one 128-wide DR
  stationary [128, 2, 64] -> both products accumulate during a SINGLE
  weight stream; PSUM [64, 512] rows 0:32 = hi, 32:64 = lo, folded with
  one vector add. Halves GEMM instruction count (512 -> 256/iter).
- Batch-sharded convs/head/tail: each core computes convs + transposes
  + zt emit only for its own 4 samples (b in [4c, 4c+4)); the fp8 z2^T
  (hi|lo) is AllGathered per iteration (2x 128KB/rank, A=k-chunks
  j%4<2, B=j%4>=2, pipelined so GEMM starts on A while B flies).
- plus_conv fp8 weight cached in SBUF (128KB/partition) across the 4
  ResPlus iterations: HBM streamed once instead of 4x.
"""
import numpy as np
import ml_dtypes

B, H, W = 32, 32, 16
HW = H * W            # 512
NC = 8                # cores
NB = B // NC          # 4 own samples per core
NFO = NB * HW         # 2048 own free elems
C = 64                # cpt channels
KCH = 256             # GEMM k-chunks of 128
OSH = 512             # output shard (4096 / 8)
HC = OSH // 2         # GEMM column half
EPS = 1e-5
PADR, PADC = H + 2, W + 2   # 34, 18
PB = PADR * PADC            # 612 per sample
PBO = NB * PB               # 2448 own padded cols

_HANDLE = {}

import os as _os
DEFAULT_CFG = (int(_os.environ.get("KRP", "4")),
               _os.environ.get("KCOLL", "1") == "1",
               _os.environ.get("KGEMM", "1") == "1",
               _os.environ.get("KCONV", "1") == "1",
               int(_os.environ.get("KWG", "16")))


def _aff(g, b, m, v):
    s = g / np.sqrt(v + EPS)
    return s.astype(np.float32), (b - m * s).astype(np.float32)


def _bf(a):
    return np.asarray(a, np.float32).astype(ml_dtypes.bfloat16)


def _diffuse2d(Wt):
    """Error-feedback fp8 quantization of W [64*HW, NO], 2-D FS per
    channel over the 32x16 spatial plane, serpentine scan."""
    q = np.empty(Wt.shape, ml_dtypes.float8_e4m3)
    NO = Wt.shape[1]
    for c in range(64):
        blk = Wt[c * HW:(c + 1) * HW].reshape(H, W, NO).astype(np.float32)
        for y in range(H):
            sgn = 1 if y % 2 == 0 else -1
            xs = range(W) if y % 2 == 0 else range(W - 1, -1, -1)
            for x in xs:
                v = blk[y, x]
                q8 = v.astype(ml_dtypes.float8_e4m3)
                e = v - q8.astype(np.float32)
                q[c * HW + y * W + x] = q8
                if 0 <= x + sgn < W:
                    blk[y, x + sgn] += e * (7 / 16)
                if y + 1 < H:
                    if 0 <= x - sgn < W:
                        blk[y + 1, x - sgn] += e * (3 / 16)
                    blk[y + 1, x] += e * (5 / 16)
                    if 0 <= x + sgn < W:
                        blk[y + 1, x + sgn] += e * (1 / 16)
    return q


def _build_nc(cfg=None):
    import concourse.bacc as bacc
    import concourse.mybir as mybir
    import concourse.tile as tile

    cfg = tuple(cfg) if cfg else DEFAULT_CFG
    if len(cfg) == 4:
        cfg = cfg + (16,)
    KRP, KCOLL, KGEMM, KCONV, WG = cfg
    f32, bf16, fp8 = mybir.dt.float32, mybir.dt.bfloat16, mybir.dt.float8e4
    DR = mybir.MatmulPerfMode.DoubleRow
    nc = bacc.Bacc("TRN2", target_bir_lowering=False, debug=False, num_devices=NC)
    RG = [list(range(NC))]

    def din(name, shape, dt=f32):
        return nc.dram_tensor(name, shape, dt, kind="ExternalInput")

    xin = {}
    xin["x_c"] = din("x_c", [6, NFO])
    xin["x_p"] = din("x_p", [8, NFO])
    xin["x_t"] = din("x_t", [8, NFO])
    xin["x_poi"] = din("x_poi", [12, NFO])
    xin["x_tm"] = din("x_tm", [31, NFO])
    for name, shape in [
        ("wcp", [14, 128]), ("wtpoi", [20, 73]), ("ones1", [1, 12]),
        ("wtm_T", [31, 28]), ("wtf_T", [28, 1]),
        ("W1a_T", [128, 64]), ("W1b_T", [73, 64]),
        ("w3p", [128, 6 * 56]), ("w4p", [128, 6 * 64]), ("W2_T", [64, 2]),
        ("ident", [64, 64]),
    ]:
        xin[name] = din(name, shape, bf16)
    for name, shape in [
        ("bcp", [128, 1]), ("btpoi", [73, 1]), ("btm", [28, 1]), ("btf", [1, 1]),
        ("b1", [64, 1]), ("sa", [64, 1]), ("ta", [64, 1]), ("sa16", [64, 1]),
        ("s56", [56, 1]), ("bz1", [56, 1]), ("s8", [8, 1]), ("t8", [8, 1]),
        ("b4", [64, 1]), ("b2", [2, 1]), ("bplus", [B, OSH]),
    ]:
        xin[name] = din(name, shape, f32)
    xin["wplus"] = din("wplus", [128, KCH, OSH], fp8)

    out_ext = nc.dram_tensor("out", [NB, 2, HW], f32, kind="ExternalOutput")

    # zt AllGather staging: slot it feeds GEMM(it). Contribution per core:
    # [128, 1024] fp8 (cols bb*256 + sb*64 + c), single-tier fp8 z2^T.
    agz_in = [nc.dram_tensor(f"agzi{i}", [128, 2048], fp8) for i in range(KRP)]
    agz_out = [nc.dram_tensor(f"agzo{i}", [NC * 128, 2048], fp8,
                              addr_space="Shared") for i in range(KRP)]
    # z2-map AllToAll (one per iteration): core j sends its OC-channel
    # rows [4k,4k+4) to core k; rows [4j,4j+4) of the output hold channel
    # j for this core's own 4 samples.
    agm_in = [nc.dram_tensor(f"agmi{i}", [B, OSH], bf16) for i in range(KRP)]
    agm_out = [nc.dram_tensor(f"agmo{i}", [B, OSH], bf16)
               for i in range(KRP)]

    Relu = mybir.ActivationFunctionType.Relu
    Tanh = mybir.ActivationFunctionType.Tanh

    with tile.TileContext(nc) as tc:
        with (
            tc.tile_pool(name="wsm", bufs=1) as wsm,
            tc.tile_pool(name="act", bufs=1) as act,
        ):
            # ---- small weights ----
            wt = {}
            for name in ["wcp", "wtpoi", "ones1", "wtm_T", "wtf_T",
                         "W1a_T", "W1b_T", "W2_T", "ident",
                         "w3p", "w4p",
                         "bcp", "btpoi", "btm", "btf", "b1", "sa", "ta", "sa16",
                         "s56", "bz1", "s8", "t8", "b4", "b2", "bplus"]:
                t = wsm.tile(list(xin[name].shape), xin[name].dtype, tag=name)
                nc.sync.dma_start(t[:], xin[name][:])
                wt[name] = t

            # ---- W cache: full per-core fp8 shard in SBUF ----
            wsb = act.tile([128, KCH * OSH], fp8, tag="wsb")
            wsbv = wsb[:].rearrange("p (j o) -> p j o", j=KCH)
            for jg in range(KCH // WG):
                nc.sync.dma_start(wsbv[:, jg * WG:(jg + 1) * WG, :],
                                  xin["wplus"].ap()[:, jg * WG:(jg + 1) * WG, :])

            # ---- head: own x cols -> cpt [64, NFO] f32 ----
            cpt = act.tile([C, NFO], f32, tag="cpt")
            with (
                tc.tile_pool(name="hx", bufs=1) as hx,
                tc.tile_pool(name="hps", bufs=1, space="PSUM") as hps,
                tc.tile_pool(name="hsb", bufs=2) as hsb,
            ):
                xcp = hx.tile([14, NFO], bf16, tag="xcp")
                xtp = hx.tile([20, NFO], bf16, tag="xtp")
                xtm = hx.tile([31, NFO], bf16, tag="x_tm")
                nc.gpsimd.dma_start(xcp[0:6, :], xin["x_c"][:])
                nc.gpsimd.dma_start(xcp[6:14, :], xin["x_p"][:])
                nc.gpsimd.dma_start(xtp[0:12, :], xin["x_poi"][:])
                nc.gpsimd.dma_start(xtp[12:20, :], xin["x_t"][:])
                nc.gpsimd.dma_start(xtm[:], xin["x_tm"][:])
                NQ = 512
                for qq in range(NFO // NQ):
                    sl = slice(qq * NQ, (qq + 1) * NQ)
                    p_tm = hps.tile([28, NQ], f32, tag="h_tm")
                    nc.tensor.matmul(p_tm[:], wt["wtm_T"][:], xtm[:, sl],
                                     start=True, stop=True)
                    tx = hsb.tile([28, NQ], bf16, tag="h_tx")
                    nc.scalar.activation(tx[:], p_tm[:], Relu, bias=wt["btm"][:])
                    p_tf = hps.tile([1, NQ], f32, tag="h_tf")
                    nc.tensor.matmul(p_tf[:], wt["wtf_T"][:], tx[:],
                                     start=True, stop=True)
                    tx2 = hsb.tile([1, NQ], bf16, tag="h_tx2")
                    nc.scalar.activation(tx2[:], p_tf[:], Relu, bias=wt["btf"][:])
                    p_bc = hps.tile([12, NQ], f32, tag="h_bc")
                    nc.tensor.matmul(p_bc[:], wt["ones1"][:], tx2[:],
                                     start=True, stop=True)
                    nc.vector.tensor_mul(xtp[0:12, sl], xtp[0:12, sl], p_bc[:])

                    p1 = hps.tile([128, NQ], f32, tag="h_p1")
                    nc.tensor.matmul(p1[:], wt["wcp"][:], xcp[:, sl],
                                     start=True, stop=True)
                    p2 = hps.tile([73, NQ], f32, tag="h_p2")
                    nc.tensor.matmul(p2[:], wt["wtpoi"][:], xtp[:, sl],
                                     start=True, stop=True)
                    y1 = hsb.tile([128, NQ], bf16, tag="h_y1")
                    nc.scalar.activation(y1[:], p1[:], Relu, bias=wt["bcp"][:])
                    y2 = hsb.tile([73, NQ], bf16, tag="h_y2")
                    nc.scalar.activation(y2[:], p2[:], Relu, bias=wt["btpoi"][:])
                    p_c1 = hps.tile([64, NQ], f32, tag="h_c1")
                    nc.tensor.matmul(p_c1[:], wt["W1a_T"][:], y1[:],
                                     start=True, stop=False)
                    nc.tensor.matmul(p_c1[:], wt["W1b_T"][:], y2[:],
                                     start=False, stop=True)
                    nc.vector.tensor_scalar_add(cpt[:, sl], p_c1[:], wt["b1"][:])

            # ---- iteration-phase pools ----
            with (
                tc.tile_pool(name="itr", bufs=1) as itr,
                tc.tile_pool(name="zb", bufs=2) as zb,
                tc.tile_pool(name="pg", bufs=1, space="PSUM") as pgp,
                tc.tile_pool(name="pcv", bufs=2, space="PSUM") as pcv,
                tc.tile_pool(name="ptr", bufs=2, space="PSUM") as ptrp,
            ):
                z1pad = itr.tile([128, PBO], bf16, tag="z1pad")
                nc.vector.memset(z1pad[:], 0.0)
                z3pad = itr.tile([128, PBO], bf16, tag="z3pad")
                nc.vector.memset(z3pad[:], 0.0)
                # gathered z2^T stationary (hi|lo tiers):
                # col = hl*8192 + b*256 + sb*64 + c ; m = hl*32 + b
                zt = itr.tile([128, 16384], fp8, tag="zt")
                sthi = itr.tile([128, 1024], fp8, tag="sthi")
                stlo = itr.tile([128, 1024], fp8, tag="stlo")
                stage = itr.tile([B, OSH], bf16, tag="stage")
                tmpf = itr.tile([B, OSH], f32, tag="tmpf")

                z1pad_v = z1pad[:].rearrange("c (b y x) -> c b y x", b=NB, y=PADR)
                z3pad_v = z3pad[:].rearrange("c (b y x) -> c b y x", b=NB, y=PADR)
                cpt_v = cpt[:].rearrange("c (b y x) -> c b y x", b=NB, y=H)
                # stationary view: [p, sb, c, m]  (m = hl*32 + b)
                ztv = zt[:].rearrange("p (m sb c) -> p sb c m", m=64, sb=4)
                sthi_v = sthi[:].rearrange("p (bb sc) -> p bb sc", bb=NB)
                stlo_v = stlo[:].rearrange("p (bb sc) -> p bb sc", bb=NB)

                def emit_sample(bb):
                    """cpt own sample bb -> fp8 z2^T staging (transposed)."""
                    sl = slice(bb * HW, (bb + 1) * HW)
                    z2h = zb.tile([C, HW], bf16, tag="z2h")
                    nc.scalar.activation(z2h[:], cpt[:, sl], Relu,
                                         scale=wt["sa16"][:])
                    ptb = ptrp.tile([128, 256], bf16, tag="tr")
                    for sb in range(4):
                        nc.tensor.transpose(
                            ptb[:, sb * 64:(sb + 1) * 64],
                            z2h[:, sb * 128:(sb + 1) * 128], wt["ident"][:])
                    nc.vector.tensor_copy(sthi_v[:, bb, :], ptb[:])
                    nc.vector.tensor_sub(stlo_v[:, bb, :], ptb[:],
                                         sthi_v[:, bb, :])

                def emit_flush(slot):
                    """staging -> ag_in DRAM + launch zt AllGather."""
                    agv = agz_in[slot].ap().rearrange("p (hl k) -> p hl k", hl=2)
                    nc.sync.dma_start(agv[:, 0], sthi[:])
                    nc.sync.dma_start(agv[:, 1], stlo[:])
                    if KCOLL:
                        nc.gpsimd.collective_compute(
                            "AllGather", mybir.AluOpType.bypass,
                            replica_groups=RG,
                            ins=[agz_in[slot][:].opt()],
                            outs=[agz_out[slot][:].opt()])

                def consume_zt(slot):
                    """ag_out -> zt tile (batch-ordered by source core)."""
                    dst = zt[:].rearrange("p (hl cc k) -> p hl cc k",
                                          hl=2, cc=NC)
                    src = agz_out[slot].ap().rearrange(
                        "(cc p) (hl k) -> p hl cc k", cc=NC, hl=2)
                    nc.sync.dma_start(dst, src)

                # head emit -> slot 0
                for bb in range(NB):
                    emit_sample(bb)
                emit_flush(0)

                for it in range(KRP):
                    # ---- conv1 (z1) for own samples ----
                    if KCONV:
                        for bb in range(NB):
                            nc.scalar.activation(
                                z1pad_v[0:64, bb, 1:33, 1:17], cpt_v[:, bb],
                                Relu, scale=wt["sa"][:], bias=wt["ta"][:])
                        nc.sync.dma_start(z1pad[64:128, 0:PBO - 1],
                                          z1pad[0:64, 1:PBO])
                        for bb in range(NB):
                            pz = pcv.tile([56, HW], f32, tag="cv")
                            for s in range(3):
                                nc.tensor.matmul(
                                    pz[:], wt["w3p"][:, s * 56:(s + 1) * 56],
                                    z1pad_v[:, bb, s:s + 32, 0:16],
                                    start=(s == 0), stop=False)
                            for s in range(3):
                                nc.tensor.matmul(
                                    pz[:], wt["w3p"][0:64, (3 + s) * 56:(4 + s) * 56],
                                    z1pad_v[0:64, bb, s:s + 32, 2:18],
                                    start=False, stop=(s == 2))
                            nc.scalar.activation(
                                z3pad_v[0:56, bb, 1:33, 1:17], pz[:],
                                Relu, scale=wt["s56"][:], bias=wt["bz1"][:])
                        nc.sync.dma_start(z3pad[64:120, 0:PBO - 1],
                                          z3pad[0:56, 1:PBO])

                    # ---- GEMM: 2 OC halves, k-chunk pairs ----
                    consume_zt(it)
                    pg = pgp.tile([64, OSH], f32, tag="g")
                    if not KGEMM:
                        nc.vector.memset(pg[:], 0.0)
                    for half in range(2):
                        csl = slice(half * HC, (half + 1) * HC)
                        for pr in range(128 if KGEMM else 0):
                            j0 = 2 * pr
                            c_i, sb_i = j0 // 4, j0 % 4
                            nc.tensor.matmul(
                                pg[:, csl], ztv[:, sb_i:sb_i + 2, c_i:c_i + 1, :],
                                wsbv[:, j0:j0 + 2, csl],
                                start=(pr == 0), stop=(pr == 127),
                                perf_mode=DR)
                        nc.vector.tensor_add(tmpf[:, csl], pg[32:64, csl],
                                             wt["bplus"][:, csl])
                        nc.vector.tensor_add(stage[:, csl], pg[0:32, csl],
                                             tmpf[:, csl])
                    nc.sync.dma_start(agm_in[it][:], stage[:])
                    if KCOLL:
                        nc.gpsimd.collective_compute(
                            "AllToAll", mybir.AluOpType.bypass,
                            replica_groups=RG,
                            ins=[agm_in[it][:].opt()],
                            outs=[agm_out[it][:].opt()])

                    # gathered z2-map -> z3pad rows (own samples only)
                    zmg = zb.tile([8, NB * HW], bf16, tag="zmg")
                    nc.sync.dma_start(
                        zmg[:].rearrange("j (b s) -> j b s", b=NB),
                        agm_out[it].ap().rearrange("(j b) s -> j b s", j=NC))
                    zmrel = zb.tile([8, NB * HW], bf16, tag="zmrel")
                    nc.scalar.activation(zmrel[:], zmg[:], Relu,
                                         scale=wt["s8"][:], bias=wt["t8"][:])
                    zmrel_v = zmrel[:].rearrange(
                        "j (b y x) -> j b y x", b=NB, y=H)
                    for bb in range(NB):
                        nc.sync.dma_start(
                            z3pad_v[56:64, bb, 1:33, 1:17], zmrel_v[:, bb])
                        nc.sync.dma_start(
                            z3pad_v[120:128, bb, 1:33, 0:16], zmrel_v[:, bb])

                    # conv2 (full spatial rows per sample) + residual
                    for bb in range(NB if KCONV else 0):
                        pc2 = pcv.tile([64, HW], f32, tag="cv")
                        for s in range(3):
                            nc.tensor.matmul(
                                pc2[:], wt["w4p"][:, s * 64:(s + 1) * 64],
                                z3pad_v[:, bb, s:s + 32, 0:16],
                                start=(s == 0), stop=False)
                        for s in range(3):
                            nc.tensor.matmul(
                                pc2[:],
                                wt["w4p"][0:64, (3 + s) * 64:(4 + s) * 64],
                                z3pad_v[0:64, bb, s:s + 32, 2:18],
                                start=False, stop=(s == 2))
                        sl = slice(bb * HW, (bb + 1) * HW)
                        nc.vector.tensor_add(cpt[:, sl], pc2[:], cpt[:, sl])
                        nc.vector.tensor_scalar_add(cpt[:, sl], cpt[:, sl],
                                                    wt["b4"][:])
                        if it + 1 < KRP:
                            emit_sample(bb)
                    if it + 1 < KRP:
                        emit_flush(it + 1)

                # ---- tail ----
                with tc.tile_pool(name="tps", bufs=2, space="PSUM") as tps:
                    for q in range(NB):
                        sl = slice(q * 512, (q + 1) * 512)
                        rq = zb.tile([C, 512], bf16, tag="z2h")
                        nc.scalar.activation(rq[:], cpt[:, sl], Relu)
                        po = tps.tile([2, 512], f32, tag="t_o")
                        nc.tensor.matmul(po[:], wt["W2_T"][:], rq[:],
                                         start=True, stop=True)
                        oq = zb.tile([2, 512], f32, tag="oq")
                        nc.scalar.activation(oq[:], po[:], Tanh, bias=wt["b2"][:])
                        nc.sync.dma_start(out_ext.ap()[q], oq[:])

    nc.compile()
    return nc


def _prep_inputs(inputs):
    """Host-side preprocessing -> list of 8 per-core input dicts."""
    ii = {k: np.asarray(v, np.float32) if np.asarray(v).dtype == np.float32
          else np.asarray(v) for k, v in inputs.items()}

    s1, t1 = _aff(ii["bn1_g"], ii["bn1_b"], ii["bn1_m"], ii["bn1_v"])
    sa, ta = _aff(ii["rp_bn1_g"], ii["rp_bn1_b"], ii["rp_bn1_m"], ii["rp_bn1_v"])
    sb_, tb_ = _aff(ii["rp_bn2_g"], ii["rp_bn2_b"], ii["rp_bn2_m"], ii["rp_bn2_v"])
    sc_, tc_ = _aff(ii["bn2_g"], ii["bn2_b"], ii["bn2_m"], ii["bn2_v"])

    conv1_2d = ii["conv1_w"][:, :, 0, 0]
    W1p = conv1_2d * s1[None, :]
    b1p = conv1_2d @ t1 + ii["conv1_b"]
    conv2_2d = ii["conv2_w"][:, :, 0, 0]
    W2p = conv2_2d * sc_[None, :]
    b2p = conv2_2d @ tc_ + ii["conv2_b"]

    # paired conv stationaries: slots 0-2 = taps (dy,0)|(dy,1) stacked on
    # partitions 0-63|64-127; slots 3-5 = taps (dy,2) on partitions 0-63.
    def pair_taps(wf, o):  # wf [O, I, 3, 3] -> [128, 6*o]
        out = np.zeros((128, 6 * o), np.float32)
        for dy in range(3):
            out[0:wf.shape[1], dy * o:(dy + 1) * o] = wf[:, :, dy, 0].T
            out[64:64 + wf.shape[1], dy * o:(dy + 1) * o] = wf[:, :, dy, 1].T
            out[0:wf.shape[1], (3 + dy) * o:(4 + dy) * o] = wf[:, :, dy, 2].T
        return out

    w3p = pair_taps(ii["rp_conv1_w"], 56)
    w4p = pair_taps(ii["rp_conv2_w"], 64)

    wcp = np.zeros((14, 128), np.float32)
    wcp[0:6, 0:64] = ii["convc_w"][:, :, 0, 0].T
    wcp[6:14, 64:128] = ii["convp_w"][:, :, 0, 0].T
    wtpoi = np.zeros((20, 73), np.float32)
    wtpoi[0:12, 64:73] = ii["poi_w"][:, :, 0, 0].T
    wtpoi[12:20, 0:64] = ii["convt_w"][:, :, 0, 0].T

    xl = ii["x"].transpose(1, 0, 2, 3).reshape(65, B * HW)
    base = {
        "wcp": _bf(wcp), "wtpoi": _bf(wtpoi), "ones1": _bf(np.ones((1, 12))),
        "wtm_T": _bf(ii["tm_w"][:, :, 0, 0].T),
        "wtf_T": _bf(ii["tf_w"][:, :, 0, 0].T),
        "W1a_T": _bf(W1p[:, :128].T),
        "W1b_T": _bf(W1p[:, 128:].T),
        "w3p": _bf(w3p), "w4p": _bf(w4p),
        "W2_T": _bf(W2p.T), "ident": _bf(np.eye(64)),
        "bcp": np.concatenate([ii["convc_b"], ii["convp_b"]])[:, None].astype(np.float32),
        "btpoi": np.concatenate([ii["convt_b"], ii["poi_b"]])[:, None].astype(np.float32),
        "btm": ii["tm_b"][:, None], "btf": ii["tf_b"][:, None],
        "b1": b1p[:, None].astype(np.float32), "sa": sa[:, None], "ta": ta[:, None],
        "sa16": (16.0 * sa)[:, None].astype(np.float32),
        "s56": sb_[:56, None],
        "bz1": (sb_[:56] * ii["rp_conv1_b"] + tb_[:56])[:, None].astype(np.float32),
        "s8": (sb_[56:] / 4096.0)[:, None].astype(np.float32),
        "t8": tb_[56:, None],
        "b4": ii["rp_conv2_b"][:, None], "b2": b2p[:, None].astype(np.float32),
    }

    plus_wf = ii["plus_w"].reshape(8 * HW, C * HW)
    W8 = _diffuse2d(plus_wf.T * 256.0)  # [64*HW, 4096] fp8
    ta_flat = np.repeat(ta, HW)
    in_maps = []
    for c in range(NC):
        m = dict(base)
        osl = slice(c * NFO, (c + 1) * NFO)
        m["x_c"] = np.ascontiguousarray(xl[0:6, osl])
        m["x_p"] = np.ascontiguousarray(xl[6:14, osl])
        m["x_t"] = np.ascontiguousarray(xl[14:22, osl])
        m["x_poi"] = np.ascontiguousarray(xl[22:34, osl])
        m["x_tm"] = np.ascontiguousarray(xl[34:65, osl])
        Wsh = plus_wf[c * OSH:(c + 1) * OSH]
        bias_eff = (ii["plus_b"][c * OSH:(c + 1) * OSH] + Wsh @ ta_flat)
        m["bplus"] = np.broadcast_to(4096.0 * bias_eff,
                                     (B, OSH)).astype(np.float32).copy()
        W8c = W8[:, c * OSH:(c + 1) * OSH]  # [32768, 512]
        m["wplus"] = np.ascontiguousarray(
            W8c.reshape(KCH, 128, OSH).transpose(1, 0, 2))
        in_maps.append(m)
    return in_maps


def _build_sharded(nc):
    import jax
    import numpy as _np
    from jax.sharding import Mesh, PartitionSpec
    from jax.experimental.shard_map import shard_map
    import concourse.mybir as mybir
    from concourse.bass2jax import (_bass_exec_p, install_neuronx_cc_hook,
                                    partition_id_tensor)

    install_neuronx_cc_hook()
    partition_name = nc.partition_id_tensor.name if nc.partition_id_tensor else None
    in_names, out_names, out_avals, zero_outs = [], [], [], []
    for alloc in nc.m.functions[0].allocations:
        if not isinstance(alloc, mybir.MemoryLocationSet):
            continue
        name = alloc.memorylocations[0].name
        if alloc.kind == "ExternalInput":
            if name != partition_name:
                in_names.append(name)
        elif alloc.kind == "ExternalOutput":
            shape = tuple(alloc.tensor_shape)
            dtype = mybir.dt.np(alloc.dtype)
            out_avals.append(jax.core.ShapedArray(shape, dtype))
            out_names.append(name)
            zero_outs.append(_np.zeros(shape, dtype))
    n_params = len(in_names)
    n_outs = len(out_avals)
    all_in_names = list(in_names) + list(out_names)
    if partition_name is not None:
        all_in_names.append(partition_name)
    donate = tuple(range(n_params, n_params + n_outs))

    def _body(*args):
        operands = list(args)
        if partition_name is not None:
            operands.append(partition_id_tensor())
        outs = _bass_exec_p.bind(
            *operands, out_avals=tuple(out_avals), in_names=tuple(all_in_names),
            out_names=tuple(out_names), lowering_input_output_aliases=(),
            sim_require_finite=True, sim_require_nnan=True, nc=nc)
        return tuple(outs)

    devices = jax.devices()[:NC]
    mesh = Mesh(_np.asarray(devices), ("core",))
    in_specs = (PartitionSpec("core"),) * (n_params + n_outs)
    out_specs = (PartitionSpec("core"),) * n_outs
    fn = jax.jit(
        shard_map(_body, mesh=mesh, in_specs=in_specs, out_specs=out_specs,
                  check_rep=False),
        donate_argnums=donate, keep_unused=True)
    return dict(fn=fn, in_names=in_names, out_names=out_names,
                out_avals=out_avals, zero_outs=zero_outs)


def get_compiled(cfg=None):
    key = tuple(cfg) if cfg else DEFAULT_CFG
    if len(key) == 4:
        key = key + (16,)
    if key not in _HANDLE:
        nc = _build_nc(key)
        _HANDLE[key] = _build_sharded(nc)
    return _HANDLE[key]


def stage_inputs(in_maps, cfg=None):
    import jax
    import numpy as _np
    from jax.sharding import Mesh, NamedSharding, PartitionSpec
    h = get_compiled(cfg)
    mesh = Mesh(_np.asarray(jax.devices()[:NC]), ("core",))
    sh = NamedSharding(mesh, PartitionSpec("core"))
    concat_in = [_np.concatenate([_np.asarray(in_maps[c][name]) for c in range(NC)],
                                 axis=0) for name in h["in_names"]]
    return [jax.device_put(a, sh) for a in concat_in]


def run_staged(staged, cfg=None):
    import jax
    import numpy as _np
    h = get_compiled(cfg)
    concat_zeros = [_np.zeros((NC * z.shape[0], *z.shape[1:]), z.dtype)
                    for z in h["zero_outs"]]
    out_arrs = h["fn"](*staged, *concat_zeros)
    jax.block_until_ready(out_arrs)
    return [
        {name: _np.asarray(out_arrs[i]).reshape(NC, *h["out_avals"][i].shape)[c]
         for i, name in enumerate(h["out_names"])}
        for c in range(NC)
    ]


def run_spmd(in_maps, cfg=None):
    import jax
    import numpy as _np
    h = get_compiled(cfg)
    concat_in = [_np.concatenate([_np.asarray(in_maps[c][name]) for c in range(NC)],
                                 axis=0) for name in h["in_names"]]
    concat_zeros = [_np.zeros((NC * z.shape[0], *z.shape[1:]), z.dtype)
                    for z in h["zero_outs"]]
    out_arrs = h["fn"](*concat_in, *concat_zeros)
    jax.block_until_ready(out_arrs)
    return [
        {name: _np.asarray(out_arrs[i]).reshape(NC, *h["out_avals"][i].shape)[c]
         for i, name in enumerate(h["out_names"])}
        for c in range(NC)
    ]


def kernel(**inputs):
    in_maps = _prep_inputs(inputs)
    results = run_spmd(in_maps)
    full = np.concatenate([results[c]["out"] for c in range(NC)], axis=0)
    return full.reshape(B, 2, H, W).astype(np.float32)


# revision 16
# speedup vs baseline: 1.3710x; 1.0334x over previous
"""DeepSTN on 8 Trainium2 NeuronCores — v4.

v2 baseline: replicated convs, OC-sharded fp8 DoubleRow GEMM with hi+lo
error-split (2 passes over the weight stream), 2 AllGathers/iter.

v4 changes (vs v2):
- Merged hi|lo stationary: hi and lo z2^T packed into one 128-wide DR
  stationary [128, 2, 64] -> both products accumulate during a SINGLE
  weight stream; PSUM [64, 512] rows 0:32 = hi, 32:64 = lo, folded with
  two vector adds. Halves GEMM instruction count (512 -> 256/iter).
- Batch-sharded convs/head/tail: each core computes convs + transposes
  + z2^T emit only for its own 4 samples (b in [4c, 4c+4)). The fp8
  hi|lo z2^T is exchanged with ONE AllGather per iteration (256KB/rank
  -> 2MB), and the z2 map with ONE AllToAll per iteration ([B, 512]
  bf16, block k -> core k). Collective count per iteration: 2 (the
  cost floor per collective call dominates; fewer+bigger is faster).
- conv2 unsplit (full 32 spatial rows per sample, 6 matmuls/sample).
- plus_conv fp8 weight cached in SBUF (128KB/partition) across the 4
  ResPlus iterations: HBM streamed once instead of 4x.
(Single-tier fp8 z2 without the lo correction was tested and rejected:
rel err 0.0198-0.0211 vs the 0.02 gate. A remote-DMA exchange replacing
the collectives is the next big lever: the cost-model charges 15us
fixed + bytes/40GBps per collective on a serialized device, ~83us/iter
of the ~99us/iter measured.)
"""
import numpy as np
import ml_dtypes

B, H, W = 32, 32, 16
HW = H * W            # 512
NC = 8                # cores
NB = B // NC          # 4 own samples per core
NFO = NB * HW         # 2048 own free elems
C = 64                # cpt channels
KCH = 256             # GEMM k-chunks of 128
OSH = 512             # output shard (4096 / 8)
HC = OSH // 2         # GEMM column half
EPS = 1e-5
PADR, PADC = H + 2, W + 2   # 34, 18
PB = PADR * PADC            # 612 per sample
PBO = NB * PB               # 2448 own padded cols

_HANDLE = {}

import os as _os
DEFAULT_CFG = (int(_os.environ.get("KRP", "4")),
               _os.environ.get("KCOLL", "1") == "1",
               _os.environ.get("KGEMM", "1") == "1",
               _os.environ.get("KCONV", "1") == "1",
               int(_os.environ.get("KWG", "16")))


def _aff(g, b, m, v):
    s = g / np.sqrt(v + EPS)
    return s.astype(np.float32), (b - m * s).astype(np.float32)


def _bf(a):
    return np.asarray(a, np.float32).astype(ml_dtypes.bfloat16)


def _diffuse2d(Wt):
    """Error-feedback fp8 quantization of W [64*HW, NO], 2-D FS per
    channel over the 32x16 spatial plane, serpentine scan."""
    q = np.empty(Wt.shape, ml_dtypes.float8_e4m3)
    NO = Wt.shape[1]
    for c in range(64):
        blk = Wt[c * HW:(c + 1) * HW].reshape(H, W, NO).astype(np.float32)
        for y in range(H):
            sgn = 1 if y % 2 == 0 else -1
            xs = range(W) if y % 2 == 0 else range(W - 1, -1, -1)
            for x in xs:
                v = blk[y, x]
                q8 = v.astype(ml_dtypes.float8_e4m3)
                e = v - q8.astype(np.float32)
                q[c * HW + y * W + x] = q8
                if 0 <= x + sgn < W:
                    blk[y, x + sgn] += e * (7 / 16)
                if y + 1 < H:
                    if 0 <= x - sgn < W:
                        blk[y + 1, x - sgn] += e * (3 / 16)
                    blk[y + 1, x] += e * (5 / 16)
                    if 0 <= x + sgn < W:
                        blk[y + 1, x + sgn] += e * (1 / 16)
    return q


def _build_nc(cfg=None):
    import concourse.bacc as bacc
    import concourse.mybir as mybir
    import concourse.tile as tile

    cfg = tuple(cfg) if cfg else DEFAULT_CFG
    if len(cfg) == 4:
        cfg = cfg + (16,)
    KRP, KCOLL, KGEMM, KCONV, WG = cfg
    f32, bf16, fp8 = mybir.dt.float32, mybir.dt.bfloat16, mybir.dt.float8e4
    DR = mybir.MatmulPerfMode.DoubleRow
    nc = bacc.Bacc("TRN2", target_bir_lowering=False, debug=False, num_devices=NC)
    RG = [list(range(NC))]

    def din(name, shape, dt=f32):
        return nc.dram_tensor(name, shape, dt, kind="ExternalInput")

    xin = {}
    xin["x_c"] = din("x_c", [6, NFO])
    xin["x_p"] = din("x_p", [8, NFO])
    xin["x_t"] = din("x_t", [8, NFO])
    xin["x_poi"] = din("x_poi", [12, NFO])
    xin["x_tm"] = din("x_tm", [31, NFO])
    for name, shape in [
        ("wcp", [14, 128]), ("wtpoi", [20, 73]), ("ones1", [1, 12]),
        ("wtm_T", [31, 28]), ("wtf_T", [28, 1]),
        ("W1a_T", [128, 64]), ("W1b_T", [73, 64]),
        ("w3p", [128, 6 * 56]), ("w4p", [128, 6 * 64]), ("W2_T", [64, 2]),
        ("ident", [64, 64]),
    ]:
        xin[name] = din(name, shape, bf16)
    for name, shape in [
        ("bcp", [128, 1]), ("btpoi", [73, 1]), ("btm", [28, 1]), ("btf", [1, 1]),
        ("b1", [64, 1]), ("sa", [64, 1]), ("ta", [64, 1]), ("sa16", [64, 1]),
        ("s56", [56, 1]), ("bz1", [56, 1]), ("s8", [8, 1]), ("t8", [8, 1]),
        ("b4", [64, 1]), ("b2", [2, 1]), ("bplus", [B, OSH]),
    ]:
        xin[name] = din(name, shape, f32)
    xin["wplus"] = din("wplus", [128, KCH, OSH], fp8)

    out_ext = nc.dram_tensor("out", [NB, 2, HW], f32, kind="ExternalOutput")

    # zt AllGather staging: slot it feeds GEMM(it). Contribution per core:
    # [128, 1024] fp8 (cols bb*256 + sb*64 + c), single-tier fp8 z2^T.
    agz_in = [nc.dram_tensor(f"agzi{i}", [128, 2048], fp8) for i in range(KRP)]
    agz_out = [nc.dram_tensor(f"agzo{i}", [NC * 128, 2048], fp8,
                              addr_space="Shared") for i in range(KRP)]
    # z2-map AllToAll (one per iteration): core j sends its OC-channel
    # rows [4k,4k+4) to core k; rows [4j,4j+4) of the output hold channel
    # j for this core's own 4 samples.
    agm_in = [nc.dram_tensor(f"agmi{i}", [B, OSH], bf16) for i in range(KRP)]
    agm_out = [nc.dram_tensor(f"agmo{i}", [B, OSH], bf16)
               for i in range(KRP)]

    Relu = mybir.ActivationFunctionType.Relu
    Tanh = mybir.ActivationFunctionType.Tanh

    with tile.TileContext(nc) as tc:
        with (
            tc.tile_pool(name="wsm", bufs=1) as wsm,
            tc.tile_pool(name="act", bufs=1) as act,
        ):
            # ---- small weights ----
            wt = {}
            for name in ["wcp", "wtpoi", "ones1", "wtm_T", "wtf_T",
                         "W1a_T", "W1b_T", "W2_T", "ident",
                         "w3p", "w4p",
                         "bcp", "btpoi", "btm", "btf", "b1", "sa", "ta", "sa16",
                         "s56", "bz1", "s8", "t8", "b4", "b2", "bplus"]:
                t = wsm.tile(list(xin[name].shape), xin[name].dtype, tag=name)
                nc.sync.dma_start(t[:], xin[name][:])
                wt[name] = t

            # ---- W cache: full per-core fp8 shard in SBUF ----
            wsb = act.tile([128, KCH * OSH], fp8, tag="wsb")
            wsbv = wsb[:].rearrange("p (j o) -> p j o", j=KCH)
            for jg in range(KCH // WG):
                nc.sync.dma_start(wsbv[:, jg * WG:(jg + 1) * WG, :],
                                  xin["wplus"].ap()[:, jg * WG:(jg + 1) * WG, :])

            # ---- head: own x cols -> cpt [64, NFO] f32 ----
            cpt = act.tile([C, NFO], f32, tag="cpt")
            with (
                tc.tile_pool(name="hx", bufs=1) as hx,
                tc.tile_pool(name="hps", bufs=1, space="PSUM") as hps,
                tc.tile_pool(name="hsb", bufs=2) as hsb,
            ):
                xcp = hx.tile([14, NFO], bf16, tag="xcp")
                xtp = hx.tile([20, NFO], bf16, tag="xtp")
                xtm = hx.tile([31, NFO], bf16, tag="x_tm")
                nc.gpsimd.dma_start(xcp[0:6, :], xin["x_c"][:])
                nc.gpsimd.dma_start(xcp[6:14, :], xin["x_p"][:])
                nc.gpsimd.dma_start(xtp[0:12, :], xin["x_poi"][:])
                nc.gpsimd.dma_start(xtp[12:20, :], xin["x_t"][:])
                nc.gpsimd.dma_start(xtm[:], xin["x_tm"][:])
                NQ = 512
                for qq in range(NFO // NQ):
                    sl = slice(qq * NQ, (qq + 1) * NQ)
                    p_tm = hps.tile([28, NQ], f32, tag="h_tm")
                    nc.tensor.matmul(p_tm[:], wt["wtm_T"][:], xtm[:, sl],
                                     start=True, stop=True)
                    tx = hsb.tile([28, NQ], bf16, tag="h_tx")
                    nc.scalar.activation(tx[:], p_tm[:], Relu, bias=wt["btm"][:])
                    p_tf = hps.tile([1, NQ], f32, tag="h_tf")
                    nc.tensor.matmul(p_tf[:], wt["wtf_T"][:], tx[:],
                                     start=True, stop=True)
                    tx2 = hsb.tile([1, NQ], bf16, tag="h_tx2")
                    nc.scalar.activation(tx2[:], p_tf[:], Relu, bias=wt["btf"][:])
                    p_bc = hps.tile([12, NQ], f32, tag="h_bc")
                    nc.tensor.matmul(p_bc[:], wt["ones1"][:], tx2[:],
                                     start=True, stop=True)
                    nc.vector.tensor_mul(xtp[0:12, sl], xtp[0:12, sl], p_bc[:])

                    p1 = hps.tile([128, NQ], f32, tag="h_p1")
                    nc.tensor.matmul(p1[:], wt["wcp"][:], xcp[:, sl],
                                     start=True, stop=True)
                    p2 = hps.tile([73, NQ], f32, tag="h_p2")
                    nc.tensor.matmul(p2[:], wt["wtpoi"][:], xtp[:, sl],
                                     start=True, stop=True)
                    y1 = hsb.tile([128, NQ], bf16, tag="h_y1")
                    nc.scalar.activation(y1[:], p1[:], Relu, bias=wt["bcp"][:])
                    y2 = hsb.tile([73, NQ], bf16, tag="h_y2")
                    nc.scalar.activation(y2[:], p2[:], Relu, bias=wt["btpoi"][:])
                    p_c1 = hps.tile([64, NQ], f32, tag="h_c1")
                    nc.tensor.matmul(p_c1[:], wt["W1a_T"][:], y1[:],
                                     start=True, stop=False)
                    nc.tensor.matmul(p_c1[:], wt["W1b_T"][:], y2[:],
                                     start=False, stop=True)
                    nc.vector.tensor_scalar_add(cpt[:, sl], p_c1[:], wt["b1"][:])

            # ---- iteration-phase pools ----
            with (
                tc.tile_pool(name="itr", bufs=1) as itr,
                tc.tile_pool(name="zb", bufs=2) as zb,
                tc.tile_pool(name="pg", bufs=1, space="PSUM") as pgp,
                tc.tile_pool(name="pcv", bufs=2, space="PSUM") as pcv,
                tc.tile_pool(name="ptr", bufs=2, space="PSUM") as ptrp,
            ):
                z1pad = itr.tile([128, PBO], bf16, tag="z1pad")
                nc.vector.memset(z1pad[:], 0.0)
                z3pad = itr.tile([128, PBO], bf16, tag="z3pad")
                nc.vector.memset(z3pad[:], 0.0)
                # gathered z2^T stationary (hi|lo tiers):
                # col = hl*8192 + b*256 + sb*64 + c ; m = hl*32 + b
                zt = itr.tile([128, 16384], fp8, tag="zt")
                sthi = itr.tile([128, 1024], fp8, tag="sthi")
                stlo = itr.tile([128, 1024], fp8, tag="stlo")
                stage = itr.tile([B, OSH], bf16, tag="stage")
                tmpf = itr.tile([B, OSH], f32, tag="tmpf")

                z1pad_v = z1pad[:].rearrange("c (b y x) -> c b y x", b=NB, y=PADR)
                z3pad_v = z3pad[:].rearrange("c (b y x) -> c b y x", b=NB, y=PADR)
                cpt_v = cpt[:].rearrange("c (b y x) -> c b y x", b=NB, y=H)
                # stationary view: [p, sb, c, m]  (m = hl*32 + b)
                ztv = zt[:].rearrange("p (m sb c) -> p sb c m", m=64, sb=4)
                sthi_v = sthi[:].rearrange("p (bb sc) -> p bb sc", bb=NB)
                stlo_v = stlo[:].rearrange("p (bb sc) -> p bb sc", bb=NB)

                def emit_sample(bb):
                    """cpt own sample bb -> fp8 z2^T staging (transposed)."""
                    sl = slice(bb * HW, (bb + 1) * HW)
                    z2h = zb.tile([C, HW], bf16, tag="z2h")
                    nc.scalar.activation(z2h[:], cpt[:, sl], Relu,
                                         scale=wt["sa16"][:])
                    ptb = ptrp.tile([128, 256], bf16, tag="tr")
                    for sb in range(4):
                        nc.tensor.transpose(
                            ptb[:, sb * 64:(sb + 1) * 64],
                            z2h[:, sb * 128:(sb + 1) * 128], wt["ident"][:])
                    nc.vector.tensor_copy(sthi_v[:, bb, :], ptb[:])
                    nc.vector.tensor_sub(stlo_v[:, bb, :], ptb[:],
                                         sthi_v[:, bb, :])

                def emit_flush(slot):
                    """staging -> ag_in DRAM + launch zt AllGather."""
                    agv = agz_in[slot].ap().rearrange("p (hl k) -> p hl k", hl=2)
                    nc.sync.dma_start(agv[:, 0], sthi[:])
                    nc.sync.dma_start(agv[:, 1], stlo[:])
                    if KCOLL:
                        nc.gpsimd.collective_compute(
                            "AllGather", mybir.AluOpType.bypass,
                            replica_groups=RG,
                            ins=[agz_in[slot][:].opt()],
                            outs=[agz_out[slot][:].opt()])

                def consume_zt(slot):
                    """ag_out -> zt tile (batch-ordered by source core)."""
                    dst = zt[:].rearrange("p (hl cc k) -> p hl cc k",
                                          hl=2, cc=NC)
                    src = agz_out[slot].ap().rearrange(
                        "(cc p) (hl k) -> p hl cc k", cc=NC, hl=2)
                    nc.sync.dma_start(dst, src)

                # head emit -> slot 0
                for bb in range(NB):
                    emit_sample(bb)
                emit_flush(0)

                for it in range(KRP):
                    # ---- conv1 (z1) for own samples ----
                    if KCONV:
                        for bb in range(NB):
                            nc.scalar.activation(
                                z1pad_v[0:64, bb, 1:33, 1:17], cpt_v[:, bb],
                                Relu, scale=wt["sa"][:], bias=wt["ta"][:])
                        nc.sync.dma_start(z1pad[64:128, 0:PBO - 1],
                                          z1pad[0:64, 1:PBO])
                        for bb in range(NB):
                            pz = pcv.tile([56, HW], f32, tag="cv")
                            for s in range(3):
                                nc.tensor.matmul(
                                    pz[:], wt["w3p"][:, s * 56:(s + 1) * 56],
                                    z1pad_v[:, bb, s:s + 32, 0:16],
                                    start=(s == 0), stop=False)
                            for s in range(3):
                                nc.tensor.matmul(
                                    pz[:], wt["w3p"][0:64, (3 + s) * 56:(4 + s) * 56],
                                    z1pad_v[0:64, bb, s:s + 32, 2:18],
                                    start=False, stop=(s == 2))
                            nc.scalar.activation(
                                z3pad_v[0:56, bb, 1:33, 1:17], pz[:],
                                Relu, scale=wt["s56"][:], bias=wt["bz1"][:])
                        nc.sync.dma_start(z3pad[64:120, 0:PBO - 1],
                                          z3pad[0:56, 1:PBO])

                    # ---- GEMM: 2 OC halves, k-chunk pairs ----
                    consume_zt(it)
                    pg = pgp.tile([64, OSH], f32, tag="g")
                    if not KGEMM:
                        nc.vector.memset(pg[:], 0.0)
                    for half in range(2):
                        csl = slice(half * HC, (half + 1) * HC)
                        for pr in range(128 if KGEMM else 0):
                            j0 = 2 * pr
                            c_i, sb_i = j0 // 4, j0 % 4
                            nc.tensor.matmul(
                                pg[:, csl], ztv[:, sb_i:sb_i + 2, c_i:c_i + 1, :],
                                wsbv[:, j0:j0 + 2, csl],
                                start=(pr == 0), stop=(pr == 127),
                                perf_mode=DR)
                        nc.vector.tensor_add(tmpf[:, csl], pg[32:64, csl],
                                             wt["bplus"][:, csl])
                        nc.vector.tensor_add(stage[:, csl], pg[0:32, csl],
                                             tmpf[:, csl])
                    nc.sync.dma_start(agm_in[it][:], stage[:])
                    if KCOLL:
                        nc.gpsimd.collective_compute(
                            "AllToAll", mybir.AluOpType.bypass,
                            replica_groups=RG,
                            ins=[agm_in[it][:].opt()],
                            outs=[agm_out[it][:].opt()])

                    # gathered z2-map -> z3pad rows (own samples only)
                    zmg = zb.tile([8, NB * HW], bf16, tag="zmg")
                    nc.sync.dma_start(
                        zmg[:].rearrange("j (b s) -> j b s", b=NB),
                        agm_out[it].ap().rearrange("(j b) s -> j b s", j=NC))
                    zmrel = zb.tile([8, NB * HW], bf16, tag="zmrel")
                    nc.scalar.activation(zmrel[:], zmg[:], Relu,
                                         scale=wt["s8"][:], bias=wt["t8"][:])
                    zmrel_v = zmrel[:].rearrange(
                        "j (b y x) -> j b y x", b=NB, y=H)
                    for bb in range(NB):
                        nc.sync.dma_start(
                            z3pad_v[56:64, bb, 1:33, 1:17], zmrel_v[:, bb])
                        nc.sync.dma_start(
                            z3pad_v[120:128, bb, 1:33, 0:16], zmrel_v[:, bb])

                    # conv2 (full spatial rows per sample) + residual
                    for bb in range(NB if KCONV else 0):
                        pc2 = pcv.tile([64, HW], f32, tag="cv")
                        for s in range(3):
                            nc.tensor.matmul(
                                pc2[:], wt["w4p"][:, s * 64:(s + 1) * 64],
                                z3pad_v[:, bb, s:s + 32, 0:16],
                                start=(s == 0), stop=False)
                        for s in range(3):
                            nc.tensor.matmul(
                                pc2[:],
                                wt["w4p"][0:64, (3 + s) * 64:(4 + s) * 64],
                                z3pad_v[0:64, bb, s:s + 32, 2:18],
                                start=False, stop=(s == 2))
                        sl = slice(bb * HW, (bb + 1) * HW)
                        nc.vector.tensor_add(cpt[:, sl], pc2[:], cpt[:, sl])
                        nc.vector.tensor_scalar_add(cpt[:, sl], cpt[:, sl],
                                                    wt["b4"][:])
                        if it + 1 < KRP:
                            emit_sample(bb)
                    if it + 1 < KRP:
                        emit_flush(it + 1)

                # ---- tail ----
                with tc.tile_pool(name="tps", bufs=2, space="PSUM") as tps:
                    for q in range(NB):
                        sl = slice(q * 512, (q + 1) * 512)
                        rq = zb.tile([C, 512], bf16, tag="z2h")
                        nc.scalar.activation(rq[:], cpt[:, sl], Relu)
                        po = tps.tile([2, 512], f32, tag="t_o")
                        nc.tensor.matmul(po[:], wt["W2_T"][:], rq[:],
                                         start=True, stop=True)
                        oq = zb.tile([2, 512], f32, tag="oq")
                        nc.scalar.activation(oq[:], po[:], Tanh, bias=wt["b2"][:])
                        nc.sync.dma_start(out_ext.ap()[q], oq[:])

    nc.compile()
    return nc


def _prep_inputs(inputs):
    """Host-side preprocessing -> list of 8 per-core input dicts."""
    ii = {k: np.asarray(v, np.float32) if np.asarray(v).dtype == np.float32
          else np.asarray(v) for k, v in inputs.items()}

    s1, t1 = _aff(ii["bn1_g"], ii["bn1_b"], ii["bn1_m"], ii["bn1_v"])
    sa, ta = _aff(ii["rp_bn1_g"], ii["rp_bn1_b"], ii["rp_bn1_m"], ii["rp_bn1_v"])
    sb_, tb_ = _aff(ii["rp_bn2_g"], ii["rp_bn2_b"], ii["rp_bn2_m"], ii["rp_bn2_v"])
    sc_, tc_ = _aff(ii["bn2_g"], ii["bn2_b"], ii["bn2_m"], ii["bn2_v"])

    conv1_2d = ii["conv1_w"][:, :, 0, 0]
    W1p = conv1_2d * s1[None, :]
    b1p = conv1_2d @ t1 + ii["conv1_b"]
    conv2_2d = ii["conv2_w"][:, :, 0, 0]
    W2p = conv2_2d * sc_[None, :]
    b2p = conv2_2d @ tc_ + ii["conv2_b"]

    # paired conv stationaries: slots 0-2 = taps (dy,0)|(dy,1) stacked on
    # partitions 0-63|64-127; slots 3-5 = taps (dy,2) on partitions 0-63.
    def pair_taps(wf, o):  # wf [O, I, 3, 3] -> [128, 6*o]
        out = np.zeros((128, 6 * o), np.float32)
        for dy in range(3):
            out[0:wf.shape[1], dy * o:(dy + 1) * o] = wf[:, :, dy, 0].T
            out[64:64 + wf.shape[1], dy * o:(dy + 1) * o] = wf[:, :, dy, 1].T
            out[0:wf.shape[1], (3 + dy) * o:(4 + dy) * o] = wf[:, :, dy, 2].T
        return out

    w3p = pair_taps(ii["rp_conv1_w"], 56)
    w4p = pair_taps(ii["rp_conv2_w"], 64)

    wcp = np.zeros((14, 128), np.float32)
    wcp[0:6, 0:64] = ii["convc_w"][:, :, 0, 0].T
    wcp[6:14, 64:128] = ii["convp_w"][:, :, 0, 0].T
    wtpoi = np.zeros((20, 73), np.float32)
    wtpoi[0:12, 64:73] = ii["poi_w"][:, :, 0, 0].T
    wtpoi[12:20, 0:64] = ii["convt_w"][:, :, 0, 0].T

    xl = ii["x"].transpose(1, 0, 2, 3).reshape(65, B * HW)
    base = {
        "wcp": _bf(wcp), "wtpoi": _bf(wtpoi), "ones1": _bf(np.ones((1, 12))),
        "wtm_T": _bf(ii["tm_w"][:, :, 0, 0].T),
        "wtf_T": _bf(ii["tf_w"][:, :, 0, 0].T),
        "W1a_T": _bf(W1p[:, :128].T),
        "W1b_T": _bf(W1p[:, 128:].T),
        "w3p": _bf(w3p), "w4p": _bf(w4p),
        "W2_T": _bf(W2p.T), "ident": _bf(np.eye(64)),
        "bcp": np.concatenate([ii["convc_b"], ii["convp_b"]])[:, None].astype(np.float32),
        "btpoi": np.concatenate([ii["convt_b"], ii["poi_b"]])[:, None].astype(np.float32),
        "btm": ii["tm_b"][:, None], "btf": ii["tf_b"][:, None],
        "b1": b1p[:, None].astype(np.float32), "sa": sa[:, None], "ta": ta[:, None],
        "sa16": (16.0 * sa)[:, None].astype(np.float32),
        "s56": sb_[:56, None],
        "bz1": (sb_[:56] * ii["rp_conv1_b"] + tb_[:56])[:, None].astype(np.float32),
        "s8": (sb_[56:] / 4096.0)[:, None].astype(np.float32),
        "t8": tb_[56:, None],
        "b4": ii["rp_conv2_b"][:, None], "b2": b2p[:, None].astype(np.float32),
    }

    plus_wf = ii["plus_w"].reshape(8 * HW, C * HW)
    W8 = _diffuse2d(plus_wf.T * 256.0)  # [64*HW, 4096] fp8
    ta_flat = np.repeat(ta, HW)
    in_maps = []
    for c in range(NC):
        m = dict(base)
        osl = slice(c * NFO, (c + 1) * NFO)
        m["x_c"] = np.ascontiguousarray(xl[0:6, osl])
        m["x_p"] = np.ascontiguousarray(xl[6:14, osl])
        m["x_t"] = np.ascontiguousarray(xl[14:22, osl])
        m["x_poi"] = np.ascontiguousarray(xl[22:34, osl])
        m["x_tm"] = np.ascontiguousarray(xl[34:65, osl])
        Wsh = plus_wf[c * OSH:(c + 1) * OSH]
        bias_eff = (ii["plus_b"][c * OSH:(c + 1) * OSH] + Wsh @ ta_flat)
        m["bplus"] = np.broadcast_to(4096.0 * bias_eff,
                                     (B, OSH)).astype(np.float32).copy()
        W8c = W8[:, c * OSH:(c + 1) * OSH]  # [32768, 512]
        m["wplus"] = np.ascontiguousarray(
            W8c.reshape(KCH, 128, OSH).transpose(1, 0, 2))
        in_maps.append(m)
    return in_maps


def _build_sharded(nc):
    import jax
    import numpy as _np
    from jax.sharding import Mesh, PartitionSpec
    from jax.experimental.shard_map import shard_map
    import concourse.mybir as mybir
    from concourse.bass2jax import (_bass_exec_p, install_neuronx_cc_hook,
                                    partition_id_tensor)

    install_neuronx_cc_hook()
    partition_name = nc.partition_id_tensor.name if nc.partition_id_tensor else None
    in_names, out_names, out_avals, zero_outs = [], [], [], []
    for alloc in nc.m.functions[0].allocations:
        if not isinstance(alloc, mybir.MemoryLocationSet):
            continue
        name = alloc.memorylocations[0].name
        if alloc.kind == "ExternalInput":
            if name != partition_name:
                in_names.append(name)
        elif alloc.kind == "ExternalOutput":
            shape = tuple(alloc.tensor_shape)
            dtype = mybir.dt.np(alloc.dtype)
            out_avals.append(jax.core.ShapedArray(shape, dtype))
            out_names.append(name)
            zero_outs.append(_np.zeros(shape, dtype))
    n_params = len(in_names)
    n_outs = len(out_avals)
    all_in_names = list(in_names) + list(out_names)
    if partition_name is not None:
        all_in_names.append(partition_name)
    donate = tuple(range(n_params, n_params + n_outs))

    def _body(*args):
        operands = list(args)
        if partition_name is not None:
            operands.append(partition_id_tensor())
        outs = _bass_exec_p.bind(
            *operands, out_avals=tuple(out_avals), in_names=tuple(all_in_names),
            out_names=tuple(out_names), lowering_input_output_aliases=(),
            sim_require_finite=True, sim_require_nnan=True, nc=nc)
        return tuple(outs)

    devices = jax.devices()[:NC]
    mesh = Mesh(_np.asarray(devices), ("core",))
    in_specs = (PartitionSpec("core"),) * (n_params + n_outs)
    out_specs = (PartitionSpec("core"),) * n_outs
    fn = jax.jit(
        shard_map(_body, mesh=mesh, in_specs=in_specs, out_specs=out_specs,
                  check_rep=False),
        donate_argnums=donate, keep_unused=True)
    return dict(fn=fn, in_names=in_names, out_names=out_names,
                out_avals=out_avals, zero_outs=zero_outs)


def get_compiled(cfg=None):
    key = tuple(cfg) if cfg else DEFAULT_CFG
    if len(key) == 4:
        key = key + (16,)
    if key not in _HANDLE:
        nc = _build_nc(key)
        _HANDLE[key] = _build_sharded(nc)
    return _HANDLE[key]


def stage_inputs(in_maps, cfg=None):
    import jax
    import numpy as _np
    from jax.sharding import Mesh, NamedSharding, PartitionSpec
    h = get_compiled(cfg)
    mesh = Mesh(_np.asarray(jax.devices()[:NC]), ("core",))
    sh = NamedSharding(mesh, PartitionSpec("core"))
    concat_in = [_np.concatenate([_np.asarray(in_maps[c][name]) for c in range(NC)],
                                 axis=0) for name in h["in_names"]]
    return [jax.device_put(a, sh) for a in concat_in]


def run_staged(staged, cfg=None):
    import jax
    import numpy as _np
    h = get_compiled(cfg)
    concat_zeros = [_np.zeros((NC * z.shape[0], *z.shape[1:]), z.dtype)
                    for z in h["zero_outs"]]
    out_arrs = h["fn"](*staged, *concat_zeros)
    jax.block_until_ready(out_arrs)
    return [
        {name: _np.asarray(out_arrs[i]).reshape(NC, *h["out_avals"][i].shape)[c]
         for i, name in enumerate(h["out_names"])}
        for c in range(NC)
    ]


def run_spmd(in_maps, cfg=None):
    import jax
    import numpy as _np
    h = get_compiled(cfg)
    concat_in = [_np.concatenate([_np.asarray(in_maps[c][name]) for c in range(NC)],
                                 axis=0) for name in h["in_names"]]
    concat_zeros = [_np.zeros((NC * z.shape[0], *z.shape[1:]), z.dtype)
                    for z in h["zero_outs"]]
    out_arrs = h["fn"](*concat_in, *concat_zeros)
    jax.block_until_ready(out_arrs)
    return [
        {name: _np.asarray(out_arrs[i]).reshape(NC, *h["out_avals"][i].shape)[c]
         for i, name in enumerate(h["out_names"])}
        for c in range(NC)
    ]


def kernel(**inputs):
    in_maps = _prep_inputs(inputs)
    results = run_spmd(in_maps)
    full = np.concatenate([results[c]["out"] for c in range(NC)], axis=0)
    return full.reshape(B, 2, H, W).astype(np.float32)
